# revision 40
# baseline (speedup 1.0000x reference)
"""EHM (SMPLX body + FLAME head + MANO hands) Bass kernel for 8 TRN2 NeuronCores.

Sharding: VERTEX sharding -- model weights (shapedirs/posedirs/lbs weights)
dominate HBM traffic, so each core owns 1/8 of the SMPLX vertices (plus the
FLAME/MANO vertices its SMPLX rows stitch in) and computes ALL B=128 batch
elements for its shard.

v2 key restructure vs v1: joint regression is linear in betas, so
J = J_reg @ (template + shapedirs @ beta) is host-precomputed as
Jdirs = J_reg @ [shapedirs | template]  (tiny: 55x3x351).  On device the
joints come from a small Jdirs @ betaT matmul -- NO AllReduce, NO dependency
of FK on the big blend-shape stage.  FK (replicated, batch-on-partitions,
vector engine) fully overlaps the shapedirs/posedirs matmul stage.
Consequences: FLAME "even" chunks, MANO J chunks, smplx J regressor slabs all
vanish; head/hand chunks don't need smplx shapedirs (their rows get
overwritten by stitching); MANO rest joints are fully host-computed (betas
are batch-constant).

Per-vertex data layout: [vertex(partition<=128), (c, b)] with c-major free dim
(col = c*128 + b).  Batch-staged data (poses, FK, A matrices): [b(part), free].
"""

import sys

sys.path.insert(0, "/opt/trn_rl_repo")

from contextlib import ExitStack

import numpy as np
import ml_dtypes

BF16NP = ml_dtypes.bfloat16

import concourse.bass as bass
import concourse.bacc as bacc
import concourse.tile as tile
import concourse.mybir as mybir
from concourse.bass_utils import run_bass_kernel_spmd

F32 = mybir.dt.float32
BF16 = mybir.dt.bfloat16
AF = mybir.ActivationFunctionType
ALU = mybir.AluOpType

# ---------------------------------------------------------------- constants
B = 128
VS, VF, VM = 10475, 5023, 778
NL = 350
NCORES = 8

SMPLX_PARENTS = np.array([-1,0,0,0,1,2,3,4,5,6,7,8,9,9,9,12,13,14,16,17,18,19,
                          15,15,15,20,25,26,20,28,29,20,31,32,20,34,35,20,37,38,
                          21,40,41,21,43,44,21,46,47,21,49,50,21,52,53])
FLAME_PARENTS = np.array([-1,0,1,1,1])
MANO_PARENTS = np.array([-1,0,1,2,0,4,5,0,7,8,0,10,11,0,13,14])

N_PLAIN, N_HEAD, N_HL, N_HR = 768, 384, 128, 128
ROWS = N_PLAIN + N_HEAD + N_HL + N_HR        # 1408
NCH = ROWS // 128                            # 11
NCH_PLAIN = 6
CH_HEAD0 = 6                                 # chunks 6,7,8 head; 9 L; 10 R
CH_HL, CH_HR = 9, 10

PD_S_K = 189
PD_F_K = 27
PD_M_K = 135

NJ_ALL = 92
OFF_S, OFF_F, OFF_L, OFF_R = 0, 55, 60, 76
NROT = 55
ROT_S0, ROT_F0, ROT_L0, ROT_R0 = 0, 22, 25, 40

BF16_INPUTS = {"w_s", "wre_f", "w_m",
               "sd_s", "pd_s_a", "pd_s_b", "sd_f", "pd_f",
               "sd_m", "pd_m_a", "pd_m_b",
               "betaT_s", "betaT_f", "betam_rep", "jd_s", "jd_f"}


def _fk_forest():
    par = np.empty(NJ_ALL, np.int64)
    par[OFF_S:OFF_S + 55] = SMPLX_PARENTS
    par[OFF_F:OFF_F + 5] = np.where(FLAME_PARENTS < 0, -1, FLAME_PARENTS + OFF_F)
    par[OFF_L:OFF_L + 16] = np.where(MANO_PARENTS < 0, -1, MANO_PARENTS + OFF_L)
    par[OFF_R:OFF_R + 16] = np.where(MANO_PARENTS < 0, -1, MANO_PARENTS + OFF_R)
    return par


def _fk_levels(par):
    depth = np.zeros(NJ_ALL, np.int64)
    for j in range(NJ_ALL):
        if par[j] >= 0:
            depth[j] = depth[par[j]] + 1
    levels = []
    for d in range(1, int(depth.max()) + 1):
        js = np.nonzero(depth == d)[0]
        runs, i = [], 0
        while i < len(js):
            j0, p0 = int(js[i]), int(par[js[i]])
            if i + 1 < len(js):
                ds = int(js[i + 1]) - j0
                ps = int(par[js[i + 1]]) - p0
            else:
                ds, ps = 1, 0
            n = 1
            while (i + n < len(js) and int(js[i + n]) == j0 + n * ds
                   and int(par[js[i + n]]) == p0 + n * ps):
                n += 1
            if n == 1:
                ds, ps = 1, 0
            runs.append((j0, ds, n, p0, ps))
            i += n
        levels.append(runs)
    return levels


def _fk_levels_split():
    """Split forest levels into smplx-only runs and flame/mano runs (the trees
    are disjoint, so the smplx chain can be processed first)."""
    levels = _fk_levels(_fk_forest())
    ls, lfm = [], []
    for runs in levels:
        rs = [r for r in runs if r[0] < 55]
        rf = [r for r in runs if r[0] >= 55]
        if rs: ls.append(rs)
        if rf: lfm.append(rf)
    return ls, lfm


# ================================================================ host prep

def _split_sizes(total, parts):
    q, r = divmod(total, parts)
    return [q + (1 if i < r else 0) for i in range(parts)]


def _pad_ids(ids, n):
    out = np.full(n, -1, np.int64)
    out[:len(ids)] = ids
    return out


def _host_prep(inp):
    f32 = np.float32
    s2f = np.asarray(inp["smplx2flame_ind"])
    head_ix = np.asarray(inp["head_index"])
    s2l = np.asarray(inp["smplx2mano_left"])
    s2r = np.asarray(inp["smplx2mano_right"])

    head_sv = s2f[head_ix]
    special = np.zeros(VS, bool)
    special[head_sv] = True
    special[s2l] = True
    special[s2r] = True
    plain_sv = np.nonzero(~special)[0]

    pl_sp = np.cumsum([0] + _split_sizes(len(plain_sv), NCORES))
    hd_sp = np.cumsum([0] + _split_sizes(len(head_ix), NCORES))
    hl_sp = np.cumsum([0] + _split_sizes(VM, NCORES))

    sd_s_np = np.asarray(inp["smplx_shapedirs"], f32)
    pd_s_np = np.asarray(inp["smplx_posedirs"], f32)
    jr_s_np = np.asarray(inp["smplx_J_regressor"], f32)
    w_s_np = np.asarray(inp["smplx_lbs_weights"], f32)
    tmpl_s = np.asarray(inp["smplx_v_template"], f32)
    sd_f_np = np.asarray(inp["flame_shapedirs"], f32)
    pd_f_np = np.asarray(inp["flame_posedirs"], f32)
    jr_f_np = np.asarray(inp["flame_J_regressor"], f32)
    w_f_np = np.asarray(inp["flame_lbs_weights"], f32)
    tmpl_f = np.asarray(inp["flame_v_template"], f32)
    re_np = np.asarray(inp["r_eyelid"], f32)
    le_np = np.asarray(inp["l_eyelid"], f32)
    sd_m_np = np.asarray(inp["mano_shapedirs"], f32)
    pd_m_np = np.asarray(inp["mano_posedirs"], f32)
    jr_m_np = np.asarray(inp["mano_J_regressor"], f32)
    w_m_np = np.asarray(inp["mano_lbs_weights"], f32)
    tmpl_m = np.asarray(inp["mano_v_template"], f32)

    aa = np.concatenate([
        np.asarray(inp["global_pose"], f32).reshape(B, 3),
        np.asarray(inp["body_pose"], f32).reshape(B, 63),
        np.asarray(inp["jaw_params"], f32).reshape(B, 3),
        np.asarray(inp["eye_pose"], f32).reshape(B, 6),
        np.asarray(inp["left_hand_pose"], f32).reshape(B, 45),
        np.asarray(inp["right_hand_pose"], f32).reshape(B, 45),
    ], axis=1)

    # ---- MANO rest joints: batch-constant -> fully host-computed ----------
    betam_vec = np.asarray(inp["mano_betas"], f32)[0]           # [10]
    vshaped_m = tmpl_m + sd_m_np @ betam_vec                    # [VM, 3]
    J_m = jr_m_np @ vshaped_m                                   # [16, 3]
    rel_m = J_m.copy()
    rel_m[1:] -= J_m[MANO_PARENTS[1:]]
    jmb_rep = np.tile(np.ascontiguousarray(J_m.T).reshape(-1), (B, 1))
    relmb_rep = np.tile(np.ascontiguousarray(rel_m.T).reshape(-1), (B, 1))

    ep = np.asarray(inp["eyelid_params"], f32)
    aux = np.concatenate([
        np.asarray(inp["head_scale"], f32)[:, None],
        np.asarray(inp["left_hand_scale"], f32)[:, None],
        np.asarray(inp["right_hand_scale"], f32)[:, None],
        ep[:, 0:1], ep[:, 1:2],
        np.asarray(inp["head_pos_offset"], f32),
        np.asarray(inp["left_hand_pos_offset"], f32) - J_m[0][None],
        np.asarray(inp["right_hand_pos_offset"], f32) - J_m[0][None],
    ], axis=1)                                               # [128, 14]

    def beta_T(second):
        b = np.concatenate([np.asarray(inp["shape_params"], f32), second], 1)
        bt = np.zeros((384, B), f32)
        bt[:NL] = b.T
        bt[NL] = 1.0
        return bt.reshape(3, 128, B)

    betaT_s = beta_T(np.asarray(inp["body_exp"], f32))
    betaT_f = beta_T(np.asarray(inp["flame_exp"], f32))

    joff = np.asarray(inp["joints_offset"], f32)
    joffT = np.ascontiguousarray(joff.transpose(1, 2, 0)).reshape(55, 384)

    # ---- precomputed joint regressor directions: J = jd . [beta;1] --------
    def jdirs(jr, sd, tmpl, nj):
        ja = (jr @ sd.reshape(-1, 3 * NL)).reshape(nj, 3, NL)   # [nj,3,350]
        jt = jr @ tmpl                                          # [nj,3]
        out = np.zeros((3, 3, 128, nj), f32)
        for c in range(3):
            full = np.zeros((384, nj), f32)
            full[:NL] = ja[:, c, :].T
            full[NL] = jt[:, c]
            out[c] = full.reshape(3, 128, nj)
        return out.reshape(9, 128, nj)

    jd_s = jdirs(jr_s_np, sd_s_np, tmpl_s, 55)
    jd_f = jdirs(jr_f_np, sd_f_np, tmpl_f, 5)

    def mrel_T(par, nj):
        m = np.eye(nj, dtype=f32)
        for j in range(1, nj):
            if par[j] >= 0:
                m[j, par[j]] = -1.0
        return np.ascontiguousarray(m.T)

    betam_rep = np.zeros((11, 128), f32)
    betam_rep[:10] = betam_vec[:, None]
    betam_rep[10] = 1.0

    bpack = np.concatenate([aa, aux, jmb_rep, relmb_rep], 1)       # [128, 275]
    spack = np.zeros((55, 444), f32)
    spack[:, 0:55] = mrel_T(SMPLX_PARENTS, 55)
    spack[0:5, 55:60] = mrel_T(FLAME_PARENTS, 5)
    spack[:, 60:444] = joffT

    rep = dict(bpack=bpack, spack=spack, betaT_s=betaT_s, betaT_f=betaT_f,
               betam_rep=betam_rep, jd_s=jd_s, jd_f=jd_f,
               ident=np.eye(128, dtype=f32))

    in_maps = []
    vid_all = np.full((NCORES, ROWS), -1, np.int64)

    for c in range(NCORES):
        p_ids = plain_sv[pl_sp[c]:pl_sp[c + 1]]
        h_pos = np.arange(hd_sp[c], hd_sp[c + 1])
        h_sv, h_fv = head_sv[h_pos], head_ix[h_pos]
        l_pos = np.arange(hl_sp[c], hl_sp[c + 1])
        r_pos = l_pos                                         # same split for R
        l_sv, r_sv = s2l[l_pos], s2r[r_pos]

        vid = np.full(ROWS, -1, np.int64)
        vid[:len(p_ids)] = p_ids
        vid[N_PLAIN:N_PLAIN + len(h_sv)] = h_sv
        vid[N_PLAIN + N_HEAD:N_PLAIN + N_HEAD + len(l_sv)] = l_sv
        vid[N_PLAIN + N_HEAD + N_HL:N_PLAIN + N_HEAD + N_HL + len(r_sv)] = r_sv
        vid_all[c] = vid
        vok = vid >= 0
        vc = np.where(vok, vid, 0)

        # smplx shapedirs slab for PLAIN chunks only: [6, 128(p=l), (c, lk, v)]
        pv, pok = vc[:N_PLAIN], vok[:N_PLAIN]
        sdp = np.zeros((N_PLAIN, 3, 384), f32)
        sdp[:, :, :NL] = np.where(pok[:, None, None], sd_s_np[pv], 0.0)
        sdp[:, :, NL] = np.where(pok[:, None], tmpl_s[pv], 0.0)
        slab = sdp.reshape(NCH_PLAIN, 128, 3, 3, 128).transpose(0, 4, 2, 3, 1)
        sd_s = np.ascontiguousarray(slab).reshape(NCH_PLAIN, 128, 1152)

        colv = vc[:, None] * 3 + np.arange(3)[None, :]
        pdv = pd_s_np[:PD_S_K][:, colv]
        pdv = np.where(vok[None, :, None], pdv, 0.0)
        pdv = pdv.reshape(PD_S_K, NCH, 128, 3).transpose(1, 0, 3, 2)
        pd_s_a = np.ascontiguousarray(pdv[:, :128]).reshape(NCH, 128, 384)
        pd_s_b = np.ascontiguousarray(pdv[:, 128:]).reshape(NCH, PD_S_K - 128, 384)

        w_s = np.ascontiguousarray(
            np.where(vok[:, None], w_s_np[vc], 0.0)
            .reshape(NCH, 128, 55).transpose(0, 2, 1))

        # flame: 3 gathered chunks (only vertices actually stitched)
        fg = _pad_ids(h_fv, N_HEAD)
        fok = fg >= 0
        fc = np.where(fok, fg, 0)
        sdfp = np.zeros((N_HEAD, 3, 384), f32)
        sdfp[:, :, :NL] = np.where(fok[:, None, None], sd_f_np[fc], 0.0)
        sdfp[:, :, NL] = np.where(fok[:, None], tmpl_f[fc], 0.0)
        slab = sdfp.reshape(-1, 128, 3, 3, 128).transpose(0, 4, 2, 3, 1)
        sd_f = np.ascontiguousarray(slab).reshape(-1, 128, 1152)

        colf = fc[:, None] * 3 + np.arange(3)[None, :]
        pdfv = pd_f_np[9:36][:, colf]
        pdfv = np.where(fok[None, :, None], pdfv, 0.0)
        pdfv = pdfv.reshape(PD_F_K, 3, 128, 3).transpose(1, 0, 3, 2)
        pd_f = np.ascontiguousarray(pdfv).reshape(3, PD_F_K, 384)

        wre = np.zeros((3, 46, 128), f32)
        for k in range(3):
            rows, ok = fc[k * 128:(k + 1) * 128], fok[k * 128:(k + 1) * 128]
            wre[k, :5] = np.where(ok[None, :], w_f_np[rows].T, 0.0)
            wre[k, 37:40] = np.where(ok[None, :], re_np[rows].T, 0.0)
            wre[k, 40:43] = np.where(ok[None, :], le_np[rows].T, 0.0)
            wre[k, 43] = 1.0                                  # bias row

        # mano hands
        m_rows = np.stack([_pad_ids(l_pos, 128), _pad_ids(r_pos, 128)])
        mok = m_rows >= 0
        mc = np.where(mok, m_rows, 0)
        sd_m = np.zeros((2, 11, 384), f32)
        pd_m_a = np.zeros((2, 128, 384), f32)
        pd_m_b = np.zeros((2, PD_M_K - 128, 384), f32)
        w_m = np.zeros((2, 46, 128), f32)
        for h in range(2):
            sdm = np.where(mok[h][:, None, None], sd_m_np[mc[h]], 0.0)
            sd_m[h, :10] = sdm.transpose(2, 1, 0).reshape(10, 384)
            sd_m[h, 10] = np.where(mok[h][:, None], tmpl_m[mc[h]], 0.0).T.reshape(384)
            colm = mc[h][:, None] * 3 + np.arange(3)[None, :]
            pdm = pd_m_np[:, colm]
            pdm = np.where(mok[h][None, :, None], pdm, 0.0).transpose(0, 2, 1)
            pd_m_a[h] = pdm[:128].reshape(128, 384)
            pd_m_b[h] = pdm[128:].reshape(PD_M_K - 128, 384)
            w_m[h, 5 + h * 16:21 + h * 16] = np.where(mok[h][None, :],
                                                      w_m_np[mc[h]].T, 0.0)
            w_m[h, 44 + h] = 1.0                              # bias row

        m = dict(rep)
        m.update(sd_s=sd_s, pd_s_a=pd_s_a, pd_s_b=pd_s_b, w_s=w_s,
                 sd_f=sd_f, pd_f=pd_f, wre_f=wre,
                 sd_m=sd_m, pd_m_a=pd_m_a, pd_m_b=pd_m_b, w_m=w_m)
        out = {}
        for k, v in m.items():
            if k in BF16_INPUTS:
                out[k] = np.ascontiguousarray(v.astype(BF16NP))
            else:
                out[k] = np.ascontiguousarray(v, f32)
        in_maps.append(out)

    return in_maps, vid_all


# ================================================================ device IR

def _build_nc():
    nc = bacc.Bacc("TRN2", target_bir_lowering=False, debug=False,
                   num_devices=NCORES)
    di = {}

    def din(name, shape):
        dt = BF16 if name in BF16_INPUTS else F32
        di[name] = nc.dram_tensor(name, list(shape), dt, kind="ExternalInput").ap()

    din("bpack", (B, 275)); din("spack", (55, 444))
    din("betaT_s", (3, 128, 128)); din("betaT_f", (3, 128, 128))
    din("betam_rep", (11, 128)); din("ident", (128, 128))
    din("jd_s", (9, 128, 55)); din("jd_f", (9, 128, 5))
    din("sd_s", (NCH_PLAIN, 128, 1152))
    din("pd_s_a", (NCH, 128, 384)); din("pd_s_b", (NCH, PD_S_K - 128, 384))
    din("w_s", (NCH, 55, 128))
    din("sd_f", (3, 128, 1152))
    din("pd_f", (3, PD_F_K, 384)); din("wre_f", (3, 46, 128))
    din("sd_m", (2, 11, 384)); din("pd_m_a", (2, 128, 384))
    din("pd_m_b", (2, PD_M_K - 128, 384)); din("w_m", (2, 46, 128))

    out_d = nc.dram_tensor("out", [ROWS, 384], BF16, kind="ExternalOutput").ap()
    dbg_d = None
    if DEBUG:
        dbg_d = nc.dram_tensor("dbg", [128, 8192], F32, kind="ExternalOutput").ap()

    with tile.TileContext(nc) as tc:
        _emit(nc, tc, di, out_d, dbg_d)
    nc.compile()
    return nc


def _emit(nc, tc, di, out_d, dbg_d=None):
    levels_s, levels_fm = _fk_levels_split()
    es = ExitStack()
    persist = es.enter_context(tc.tile_pool(name="persist", bufs=1))
    slabs = es.enter_context(tc.tile_pool(name="slabs", bufs=3))
    acc_cm = tc.tile_pool(name="acc", bufs=2, space="PSUM")
    acc = acc_cm.__enter__()
    big_cm = tc.tile_pool(name="big", bufs=2, space="PSUM")
    big = big_cm.__enter__()

    V, S, G, T, DMA = nc.vector, nc.scalar, nc.gpsimd, nc.tensor, nc.sync

    def ptile(shape, name):
        return persist.tile(list(shape), F32, tag=name, name=name)

    def btile(shape, name):
        return persist.tile(list(shape), BF16, tag=name, name=name)

    # ---------------- staged inputs ---------------------------------------
    # Each dma_start costs ~1.4us of issue time on its queue, so inputs are
    # packed into few transfers and spread over the three DGE queues
    # (Sync / Scalar / GpSimd) with the critical path (aa -> rodrigues,
    # jds/betaT -> joints -> FK) first on Sync.
    bpack = ptile((B, 275), "bpack")
    G.dma_start(bpack[:], di["bpack"][:])
    aa = bpack[:, 0:165]
    aux = bpack[:, 165:179]
    jmb = bpack[:, 179:227]
    relmb = bpack[:, 227:275]
    jds = btile((128, 9 * 55), "jds")
    DMA.dma_start(jds[:].rearrange("p (k j) -> p k j", j=55),
                  di["jd_s"][:].rearrange("k p j -> p k j"))
    betaT_s = btile((128, 384), "betaT_s")
    DMA.dma_start(betaT_s[:].rearrange("p (k b) -> p k b", b=128),
                  di["betaT_s"][:].rearrange("k p b -> p k b"))
    ident = ptile((128, 128), "ident")
    DMA.dma_start(ident[:], di["ident"][:])
    # flame-path inputs off the Sync queue (flame J is not start-critical)
    jdf = btile((128, 9 * 5), "jdf")
    S.dma_start(jdf[:].rearrange("p (k j) -> p k j", j=5),
                di["jd_f"][:].rearrange("k p j -> p k j"))
    betaT_f = btile((128, 384), "betaT_f")
    S.dma_start(betaT_f[:].rearrange("p (k b) -> p k b", b=128),
                di["betaT_f"][:].rearrange("k p b -> p k b"))
    spack = ptile((55, 444), "spack")
    G.dma_start(spack[:], di["spack"][:])
    mrelT_s = spack[:, 0:55]
    mrelT_f = spack[0:5, 55:60]
    joffT = spack[:, 60:444]
    betam = btile((11, 128), "betam")
    G.dma_start(betam[:], di["betam_rep"][:])

    # ---------------- rodrigues (V + S) -----------------------------------
    rot = ptile((B, NROT * 9), "rot")
    _rodrigues(nc, aa, rot, ptile)
    rot4 = rot[:].rearrange("p (j x) -> p j x", x=9)

    # skinning weights preloaded via the Scalar DGE queue (after rodrigues
    # so its activations are not delayed)
    w_all = persist.tile([55, NCH * 128], BF16, tag="w_all", name="w_all")
    S.dma_start(w_all[:].rearrange("j (i b) -> j i b", b=128),
                di["w_s"][:].rearrange("i j b -> j i b"))
    wre_all = persist.tile([46, 384], BF16, tag="wre_all", name="wre_all")
    S.dma_start(wre_all[:].rearrange("r (h b) -> r h b", b=128),
                di["wre_f"][:].rearrange("h r b -> r h b"))
    wm_all = persist.tile([46, 256], BF16, tag="wm_all", name="wm_all")
    S.dma_start(wm_all[:].rearrange("r (h b) -> r h b", b=128),
                di["w_m"][:].rearrange("h r b -> r h b"))

    # ---------------- joints from betas (tensor, tiny) ---------------------
    jp = acc.tile([128, 384], F32, tag="acc", padded_shape=[128, 512])
    for c3 in range(3):
        for lk in range(3):
            T.matmul(jp[0:55, c3 * 128:(c3 + 1) * 128],
                     jds[:, (c3 * 3 + lk) * 55:(c3 * 3 + lk + 1) * 55],
                     betaT_s[:, lk * 128:(lk + 1) * 128],
                     start=(lk == 0), stop=(lk == 2))
    tbj = ptile((55, 384), "tbj")
    V.tensor_add(tbj[:], jp[0:55, :], joffT)

    jpf = acc.tile([128, 384], F32, tag="acc", padded_shape=[128, 512])
    for c3 in range(3):
        for lk in range(3):
            T.matmul(jpf[0:5, c3 * 128:(c3 + 1) * 128],
                     jdf[:, (c3 * 3 + lk) * 5:(c3 * 3 + lk + 1) * 5],
                     betaT_f[:, lk * 128:(lk + 1) * 128],
                     start=(lk == 0), stop=(lk == 2))
    arr_f = ptile((5, 384), "arr_f")
    S.copy(arr_f[:], jpf[0:5, :])

    # rel joints
    rel_s = ptile((55, 384), "rel_s")
    pp = acc.tile([128, 384], F32, tag="acc", padded_shape=[128, 512])
    T.matmul(pp[0:55, :], mrelT_s, tbj[:], start=True, stop=True)
    S.copy(rel_s[:], pp[0:55, :])
    rel_f = ptile((5, 384), "rel_f")
    pp = acc.tile([128, 384], F32, tag="acc", padded_shape=[128, 512])
    T.matmul(pp[0:5, :], mrelT_f, arr_f[:], start=True, stop=True)
    S.copy(rel_f[:], pp[0:5, :])

    def transpose_to(dst_ap, src_ap):
        pq = acc.tile([128, 384], F32, tag="acc", padded_shape=[128, 512])
        k, n = src_ap.shape[0], src_ap.shape[1]
        T.matmul(pq[:n, :k], src_ap, ident[:k, :k], is_transpose=True,
                 start=True, stop=True)
        S.copy(dst_ap, pq[:n, :k])

    # batch-major staging of joints / rel for FK
    jb = ptile((B, 165), "jb")
    relb = ptile((B, 165), "relb")
    jfb = ptile((B, 15), "jfb")
    relfb = ptile((B, 15), "relfb")
    for c3 in range(3):
        transpose_to(jb[:, c3 * 55:(c3 + 1) * 55], tbj[:, c3 * 128:(c3 + 1) * 128])
        transpose_to(relb[:, c3 * 55:(c3 + 1) * 55], rel_s[:, c3 * 128:(c3 + 1) * 128])
        transpose_to(jfb[:, c3 * 5:(c3 + 1) * 5], arr_f[:, c3 * 128:(c3 + 1) * 128])
        transpose_to(relfb[:, c3 * 5:(c3 + 1) * 5], rel_f[:, c3 * 128:(c3 + 1) * 128])

    # pf = rot - I staged for posedirs matmuls (transposed, bf16)
    def pf_make(name, j0, n):
        t = ptile((B, n * 9), name)
        t9 = t[:].rearrange("p (j x) -> p j x", x=9)
        V.tensor_copy(t9, rot4[:, j0:j0 + n, :])
        V.tensor_scalar_add(t9[:, :, 0:9:4], t9[:, :, 0:9:4], -1.0)
        return t

    pf_s = pf_make("pf_s", 1, 21)
    pf_f = pf_make("pf_f", 22, 3)
    pf_m = [pf_make("pf_l", 25, 15), pf_make("pf_r", 40, 15)]

    pfT_s_a = btile((128, 128), "pfT_s_a")
    pfT_s_b = btile((PD_S_K - 128, 128), "pfT_s_b")
    transpose_to(pfT_s_a[:], pf_s[:, 0:128])
    transpose_to(pfT_s_b[:], pf_s[:, 128:PD_S_K])
    pfT_f = btile((PD_F_K, 128), "pfT_f")
    transpose_to(pfT_f[:], pf_f[:, :])
    pfT_m_a = [btile((128, 128), "pfT_l_a"), btile((128, 128), "pfT_r_a")]
    pfT_m_b = [btile((PD_M_K - 128, 128), "pfT_l_b"),
               btile((PD_M_K - 128, 128), "pfT_r_b")]
    for h in range(2):
        transpose_to(pfT_m_a[h][:], pf_m[h][:, 0:128])
        transpose_to(pfT_m_b[h][:], pf_m[h][:, 128:PD_M_K])

    # ---------------- FK (vector, batch on partitions) ---------------------
    Tb = ptile((B, NJ_ALL * 12), "Tb")
    Ab = ptile((B, NJ_ALL * 12), "Ab")
    T4 = Tb[:].rearrange("p (j m n) -> p j m n", m=3, n=4)
    A4 = Ab[:].rearrange("p (j m n) -> p j m n", m=3, n=4)
    G.memset(Tb[:], 0.0)
    for j0, n in ((22, 33), (OFF_F, 2), (OFF_L, 1), (OFF_R, 1)):
        G.memset(Tb[:].rearrange("p (j x) -> p j x", x=12)[:, j0:j0 + n, 0:11:5], 1.0)
    # combined flame+mano rhs (joints 55..91 are contiguous in the forest):
    # rows 0:5 flame, 5:21 manoL, 21:37 manoR, 37:40 r_eyelid, 40:43 l_eyelid,
    # 43 head-bias ones, 44/45 hand-bias ones
    rhs_fm = persist.tile([46, 1536], BF16, tag="rhs_fm", name="rhs_fm")
    G.memset(rhs_fm[:], 0.0)

    def rot_to_T(tj0, rj0, n):
        V.tensor_copy(T4[:, tj0:tj0 + n, :, 0:3],
                      rot4[:, rj0:rj0 + n, :].rearrange("p j (m n) -> p j m n", n=3))

    def fk_run(runs_list):
        fk_tmp2 = ptile((B, 12 * 16), "fk_tmp2")
        for runs in runs_list:
            for (d0, ds, n, p0, ps) in runs:
                sl_d = slice(d0, d0 + (n - 1) * ds + 1, ds) if ds != 1 else slice(d0, d0 + n)
                dst, dT = A4[:, sl_d], T4[:, sl_d]
                if ps == 0:
                    par = A4[:, p0:p0 + 1].broadcast_to([B, n, 3, 4])
                else:
                    sl_p = slice(p0, p0 + (n - 1) * ps + 1, ps) if ps != 1 else slice(p0, p0 + n)
                    par = A4[:, sl_p]
                sc2 = fk_tmp2[:].rearrange("p (j m n) -> p j m n", m=3, n=4)[:, :n]
                for k in range(3):
                    a_k = par[:, :, :, k:k + 1].broadcast_to([B, n, 3, 4])
                    t_k = dT[:, :, k:k + 1, :].broadcast_to([B, n, 3, 4])
                    if k == 0:
                        V.tensor_mul(dst, a_k, t_k)
                    else:
                        V.tensor_mul(sc2, a_k, t_k)
                        V.tensor_add(dst, dst, sc2)
                V.tensor_add(dst[:, :, :, 3], dst[:, :, :, 3], par[:, :, :, 3])

    corr_tmp = ptile((B, NJ_ALL * 3), "corr_tmp")
    corr_tmp2 = ptile((B, NJ_ALL * 3), "corr_tmp2")

    def corr(j0, nj, jsrc):
        ct = corr_tmp[:].rearrange("p (j m) -> p j m", m=3)[:, 0:nj]
        ct2 = corr_tmp2[:].rearrange("p (j m) -> p j m", m=3)[:, 0:nj]
        js = jsrc.rearrange("p (c j) -> p c j", c=3)
        for k in range(3):
            a_k = A4[:, j0:j0 + nj, :, k]
            j_k = js[:, k, :].unsqueeze(2).broadcast_to([B, nj, 3])
            if k == 0:
                V.tensor_mul(ct, a_k, j_k)
            else:
                V.tensor_mul(ct2, a_k, j_k)
                V.tensor_add(ct, ct, ct2)
        V.tensor_sub(A4[:, j0:j0 + nj, :, 3], A4[:, j0:j0 + nj, :, 3], ct)

    # ---- smplx chain first: fills, levels, corr --------------------------
    rot_to_T(0, ROT_S0, 22)
    V.tensor_copy(T4[:, 0:55, :, 3], relb[:].rearrange("p (c j) -> p j c", c=3))
    V.tensor_copy(A4[:, 0:1], T4[:, 0:1])
    fk_run(levels_s)
    corr(OFF_S, 55, jb[:])

    # ================= stage A part 1: plain + hand chunks =================
    vp_sbuf = [btile((128, 384), f"vp{i}") for i in range(NCH)]
    vpf_sbuf = [btile((128, 384), f"vpf{h}") for h in range(3)]
    vpm_sbuf = [btile((128, 384), f"vpm{h}") for h in range(2)]

    def copy_vp(dst_t, pq3):
        S.copy(dst_t[:, 0:384].rearrange("p (c b) -> p c b", b=128),
               pq3[:].rearrange("p (c x) -> p c x", x=512)[:, :, 0:128])

    def stage_a_chunk(i, pda=None, pdb=None):
        pq3 = big.tile([128, 1536], F32, tag="bigp")
        sdt = None
        if i < NCH_PLAIN:
            sdt = slabs.tile((128, 1152), BF16, tag="sd_s")
            DMA.dma_start(sdt[:], di["sd_s"][i])
        if pda is None:
            pda = slabs.tile((128, 384), BF16, tag="pd_s_a")
            pdb = slabs.tile((PD_S_K - 128, 384), BF16, tag="pd_s_b")
            DMA.dma_start(pda[:], di["pd_s_a"][i])
            DMA.dma_start(pdb[:], di["pd_s_b"][i])
        for c3 in range(3):
            r = slice(c3 * 512, c3 * 512 + 128)
            if sdt is not None:
                for lk in range(3):
                    T.matmul(pq3[:, r],
                             sdt[:, (c3 * 3 + lk) * 128:(c3 * 3 + lk + 1) * 128],
                             betaT_s[:, lk * 128:(lk + 1) * 128],
                             start=(lk == 0), stop=False)
        for c3 in range(3):
            r = slice(c3 * 512, c3 * 512 + 128)
            T.matmul(pq3[:, r], pda[:, c3 * 128:(c3 + 1) * 128], pfT_s_a[:],
                     start=(sdt is None), stop=False)
        for c3 in range(3):
            r = slice(c3 * 512, c3 * 512 + 128)
            T.matmul(pq3[:, r], pdb[:, c3 * 128:(c3 + 1) * 128], pfT_s_b[:],
                     start=False, stop=True)
        if i in (CH_HL, CH_HR):
            h = i - CH_HL
            sdm = slabs.tile((11, 384), BF16, tag="sd_m")
            DMA.dma_start(sdm[:], di["sd_m"][h])
            pma = slabs.tile((128, 384), BF16, tag="pd_m_a")
            pmb = slabs.tile((PD_M_K - 128, 384), BF16, tag="pd_m_b")
            DMA.dma_start(pma[:], di["pd_m_a"][h])
            DMA.dma_start(pmb[:], di["pd_m_b"][h])
            pq2 = big.tile([128, 1536], F32, tag="bigp")
            for c3 in range(3):
                T.matmul(pq2[:, c3 * 512:c3 * 512 + 128],
                         sdm[:, c3 * 128:(c3 + 1) * 128], betam[:],
                         start=True, stop=False)
            for c3 in range(3):
                T.matmul(pq2[:, c3 * 512:c3 * 512 + 128],
                         pma[:, c3 * 128:(c3 + 1) * 128], pfT_m_a[h][:],
                         start=False, stop=False)
            for c3 in range(3):
                T.matmul(pq2[:, c3 * 512:c3 * 512 + 128],
                         pmb[:, c3 * 128:(c3 + 1) * 128], pfT_m_b[h][:],
                         start=False, stop=True)
            copy_vp(vpm_sbuf[h], pq2)
        copy_vp(vp_sbuf[i], pq3)

    # preload head/flame slabs via the (idle) GpSimd DGE queue so those
    # matmuls aren't gated by the backed-up Sync queue
    sdf_t, pdf_t, hpd_a, hpd_b = [], [], [], []
    for h in range(3):
        t = persist.tile([128, 1152], BF16, tag=f"sdf{h}", name=f"sdf{h}")
        G.dma_start(t[:], di["sd_f"][h]); sdf_t.append(t)
        t = persist.tile([PD_F_K, 384], BF16, tag=f"pdf{h}", name=f"pdf{h}")
        G.dma_start(t[:], di["pd_f"][h]); pdf_t.append(t)
        i = CH_HEAD0 + h
        t = persist.tile([128, 384], BF16, tag=f"hpa{h}", name=f"hpa{h}")
        G.dma_start(t[:], di["pd_s_a"][i]); hpd_a.append(t)
        t = persist.tile([PD_S_K - 128, 384], BF16, tag=f"hpb{h}", name=f"hpb{h}")
        G.dma_start(t[:], di["pd_s_b"][i]); hpd_b.append(t)

    for i in list(range(NCH_PLAIN)) + [CH_HL, CH_HR]:
        stage_a_chunk(i)
    for h in range(3):
        stage_a_chunk(CH_HEAD0 + h, hpd_a[h], hpd_b[h])

    # ---- rhs_s (only needs the smplx chain) -------------------------------
    def rhs_fill(rhs_t, j0, nj):
        bp = big.tile([128, 1536], F32, tag="bigp")
        for n4 in range(4):
            for m3 in range(3):
                T.matmul(bp[0:nj, n4 * 384 + m3 * 128:n4 * 384 + (m3 + 1) * 128],
                         A4[:, j0:j0 + nj, m3, n4], ident[:],
                         is_transpose=True, start=True, stop=True)
        S.copy(rhs_t[0:nj, 0:1536], bp[0:nj, :])

    rhs_s = persist.tile([55, 1536], BF16, tag="rhs_s", name="rhs_s")
    rhs_fill(rhs_s, 0, 55)

    # ---- flame/mano chains ------------------------------------------------
    rot_to_T(OFF_F + 2, ROT_F0, 3)
    rot_to_T(OFF_L + 1, ROT_L0, 15)
    rot_to_T(OFF_R + 1, ROT_R0, 15)
    V.tensor_copy(T4[:, OFF_F:OFF_F + 5, :, 3],
                  relfb[:].rearrange("p (c j) -> p j c", c=3))
    for off in (OFF_L, OFF_R):
        V.tensor_copy(T4[:, off:off + 16, :, 3],
                      relmb.rearrange("p (c j) -> p j c", c=3))
    for r in (OFF_F, OFF_L, OFF_R):
        V.tensor_copy(A4[:, r:r + 1], T4[:, r:r + 1])
    fk_run(levels_fm)

    # ---- per-batch staging (world translations BEFORE rel-correction) ----
    hm = ptile((B, 16), "hm")
    jb3 = jb[:].rearrange("p (c j) -> p c j", c=3)
    bias9 = ptile((B, 9), "bias9")
    V.tensor_add(hm[:, 0:3], jb3[:, :, 23], jb3[:, :, 24])
    V.tensor_add(hm[:, 3:6], A4[:, OFF_F + 3, :, 3], A4[:, OFF_F + 4, :, 3])
    V.tensor_sub(hm[:, 6:9], hm[:, 0:3], hm[:, 3:6])
    V.scalar_tensor_tensor(bias9[:, 0:3], hm[:, 6:9], 0.5, aux[:, 5:8],
                           ALU.mult, ALU.add)
    V.tensor_sub(bias9[:, 3:4], jb3[:, 0:1, 20], aux[:, 8:9])
    V.tensor_add(bias9[:, 4:6], aux[:, 9:11], jb3[:, 1:3, 20])
    V.tensor_add(bias9[:, 6:9], aux[:, 11:14], jb3[:, :, 21])
    epp = ptile((B, 2), "epp")
    V.tensor_mul(epp[:], aux[:, 3:5], aux[:, 0:1].broadcast_to([B, 2]))

    corr(OFF_F, 5, jfb[:])
    corr(OFF_L, 16, jmb)
    corr(OFF_R, 16, jmb)

    # ---- scale folding --------------------------------------------------
    V.tensor_scalar_mul(Ab[:, OFF_F * 12:(OFF_F + 5) * 12],
                        Ab[:, OFF_F * 12:(OFF_F + 5) * 12], aux[:, 0:1])
    negls = ptile((B, 1), "negls")
    V.tensor_scalar_mul(negls[:], aux[:, 1:2], -1.0)
    AL = A4[:, OFF_L:OFF_L + 16]
    V.tensor_scalar_mul(AL[:, :, 0, :], AL[:, :, 0, :], negls[:, 0:1])
    V.tensor_scalar_mul(AL[:, :, 1:3, :], AL[:, :, 1:3, :], aux[:, 1:2])
    ARr = A4[:, OFF_R:OFF_R + 16]
    V.tensor_scalar_mul(ARr[:, :, :, :], ARr[:, :, :, :], aux[:, 2:3])

    # ================= skinning: plain chunks ==============================
    scr_t = [btile((128, 384), f"scr{i}") for i in range(4)]
    gscr = [btile((128, 384), f"gscr{i}") for i in range(2)]

    def t_apply(E, dst_ap, tp_ap, x_t, scratch):
        """dst = sum_{n<3} T'[n]*x_n + T'[3]; layouts (n, m, b)."""
        d3 = dst_ap.rearrange("p (m b) -> p m b", b=128)
        x3 = x_t[:, 0:384].rearrange("p (c b) -> p c b", b=128)
        tp = tp_ap.rearrange("p (n m b) -> p n m b", m=3, b=128)
        sc = scratch.rearrange("p (m b) -> p m b", b=128)
        E.tensor_mul(d3, tp[:, 0], x3[:, 0:1].broadcast_to([128, 3, 128]))
        for n4 in (1, 2):
            E.tensor_mul(sc, tp[:, n4], x3[:, n4:n4 + 1].broadcast_to([128, 3, 128]))
            E.tensor_add(d3, d3, sc)
        E.tensor_add(d3, d3, tp[:, 3])

    def skin_chunk(i):
        tps = big.tile([128, 1536], F32, tag="bigp")
        for g in range(3):
            T.matmul(tps[:, g * 512:(g + 1) * 512],
                     w_all[:, i * 128:(i + 1) * 128],
                     rhs_s[:, g * 512:(g + 1) * 512], start=True, stop=True)
        tpb = slabs.tile((128, 1536), BF16, tag="tpb", bufs=3, name="tpb")
        S.copy(tpb[:], tps[:])
        ot = slabs.tile((128, 384), BF16, tag="outt", bufs=3, name="ot")
        t_apply(V, ot[:], tpb[:], vp_sbuf[i], scr_t[i % 4][:])
        DMA.dma_start(out_d[i * 128:(i + 1) * 128, :], ot[:])

    for i in range(NCH_PLAIN):
        skin_chunk(i)

    # ================= rhs_f / rhs_m =======================================
    epT = persist.tile([2, 128], BF16, tag="epT", name="epT")
    transpose_to(epT[:], epp[:, :])
    bias9T = persist.tile([9, 128], BF16, tag="bias9T", name="bias9T")
    transpose_to(bias9T[:], bias9[:, :])
    rhs_fill(rhs_fm, OFF_F, 37)
    for m3 in range(3):
        G.dma_start(rhs_fm[37 + m3:38 + m3, (9 + m3) * 128:(10 + m3) * 128],
                    epT[1:2, :])
        G.dma_start(rhs_fm[40 + m3:41 + m3, (9 + m3) * 128:(10 + m3) * 128],
                    epT[0:1, :])
        G.dma_start(rhs_fm[43:44, (9 + m3) * 128:(10 + m3) * 128],
                    bias9T[m3:m3 + 1, :])
    for h in range(2):
        for m3 in range(3):
            G.dma_start(rhs_fm[44 + h:45 + h, (9 + m3) * 128:(10 + m3) * 128],
                        bias9T[3 + 3 * h + m3:4 + 3 * h + m3, :])

    # ================= skinning: head + hand chunks ========================
    # pre-skin (flame/mano LBS) offloaded to GpSimd from a Scalar-copied
    # SBUF image of the PSUM tile, overlapping the Vector final applies
    def pre_skin(i):
        tpx = big.tile([128, 1536], F32, tag="bigp")
        if i < CH_HEAD0 + 3 and i >= CH_HEAD0:
            h = i - CH_HEAD0
            wsl, x_t = wre_all[:, h * 128:(h + 1) * 128], vpf_sbuf[h]
        else:
            h = i - CH_HL
            wsl, x_t = wm_all[:, h * 128:(h + 1) * 128], vpm_sbuf[h]
        rhs_x = rhs_fm
        for g in range(3):
            T.matmul(tpx[:, g * 512:(g + 1) * 512], wsl,
                     rhs_x[:, g * 512:(g + 1) * 512], start=True, stop=True)
        tpxb = slabs.tile((128, 1536), BF16, tag="tpb", bufs=3, name="tpb")
        S.copy(tpxb[:], tpx[:])
        hv = slabs.tile((128, 384), BF16, tag="hv", bufs=2, name="hv")
        t_apply(V, hv[:], tpxb[:], x_t, gscr[i % 2][:])
        G.tensor_add(vp_sbuf[i][:, 0:384], vp_sbuf[i][:, 0:384], hv[:])
        skin_chunk(i)

    for i in (CH_HL, CH_HR):
        pre_skin(i)

    # flame stage-A (only gates the head chunks; runs while hands finish)
    for h in range(3):
        pq3 = big.tile([128, 1536], F32, tag="bigp")
        sdt = sdf_t[h]
        pdf = pdf_t[h]
        for c3 in range(3):
            r = slice(c3 * 512, c3 * 512 + 128)
            for lk in range(3):
                T.matmul(pq3[:, r],
                         sdt[:, (c3 * 3 + lk) * 128:(c3 * 3 + lk + 1) * 128],
                         betaT_f[:, lk * 128:(lk + 1) * 128],
                         start=(lk == 0), stop=False)
        for c3 in range(3):
            r = slice(c3 * 512, c3 * 512 + 128)
            T.matmul(pq3[:, r], pdf[:, c3 * 128:(c3 + 1) * 128], pfT_f[:],
                     start=False, stop=True)
        copy_vp(vpf_sbuf[h], pq3)

    for i in range(CH_HEAD0, CH_HEAD0 + 3):
        pre_skin(i)

    if dbg_d is not None:
        DMA.dma_start(dbg_d[0:128, 0:495], rot[:])
        DMA.dma_start(dbg_d[0:128, 512:1616], Ab[:])
        DMA.dma_start(dbg_d[0:128, 1664:1829], jb[:])
        DMA.dma_start(dbg_d[0:128, 1856:2021], relb[:])
        DMA.dma_start(dbg_d[0:128, 3200:3209], bias9[:])
        DMA.dma_start(dbg_d[0:128, 3216:3232], hm[:])
        DMA.dma_start(dbg_d[0:55, 3712:4096], tbj[:])
        DMA.dma_start(dbg_d[0:55, 4096:4480], rel_s[:])
        DMA.dma_start(dbg_d[0:5, 4480:4864], arr_f[:])
        DMA.dma_start(dbg_d[0:5, 4864:5248], rel_f[:])
        DMA.dma_start(dbg_d[0:128, 5376:6480], Tb[:])
    big_cm.__exit__(None, None, None)
    acc_cm.__exit__(None, None, None)
    es.close()


def _rodrigues(nc, aa, rot, ptile):
    V, S = nc.vector, nc.scalar
    J = NROT
    aa3 = aa[:].rearrange("p (j k) -> p j k", k=3)
    sq = ptile((B, J), "rg_sq")
    tmp = ptile((B, J), "rg_tmp")
    V.tensor_mul(sq[:], aa3[:, :, 0], aa3[:, :, 0])
    V.tensor_mul(tmp[:], aa3[:, :, 1], aa3[:, :, 1])
    V.tensor_add(sq[:], sq[:], tmp[:])
    V.tensor_mul(tmp[:], aa3[:, :, 2], aa3[:, :, 2])
    V.tensor_add(sq[:], sq[:], tmp[:])
    eps_t = ptile((B, 1), "rg_eps")
    nc.gpsimd.memset(eps_t[:], 1e-8)
    hpi_t = ptile((B, 1), "rg_hpi")
    nc.gpsimd.memset(hpi_t[:], float(np.pi / 2))
    zero_t = ptile((B, 1), "rg_zero")
    nc.gpsimd.memset(zero_t[:], 0.0)
    ang = ptile((B, J), "rg_ang")
    S.activation(ang[:], sq[:], AF.Sqrt, bias=eps_t[:])
    inv = ptile((B, J), "rg_inv")
    V.reciprocal(inv[:], ang[:])
    sn = ptile((B, J), "rg_sin")
    co = ptile((B, J), "rg_cos")
    S.activation(sn[:], ang[:], AF.Sin, bias=zero_t[:])
    S.activation(co[:], ang[:], AF.Sin, bias=hpi_t[:])
    nv = ptile((B, 3 * J), "rg_n")
    n3 = nv[:].rearrange("p (j k) -> p j k", k=3)
    V.tensor_mul(n3, aa3, inv[:].unsqueeze(2).broadcast_to([B, J, 3]))
    u = ptile((B, J), "rg_u")
    V.tensor_scalar(u[:], co[:], -1.0, 1.0, ALU.mult, ALU.add)
    un = ptile((B, 3 * J), "rg_un")
    un3 = un[:].rearrange("p (j k) -> p j k", k=3)
    V.tensor_mul(un3, n3, u[:].unsqueeze(2).broadcast_to([B, J, 3]))
    q = ptile((B, 3 * J), "rg_q")
    q3 = q[:].rearrange("p (j k) -> p j k", k=3)
    V.tensor_mul(q3, un3, n3)
    d = ptile((B, J), "rg_d")
    V.tensor_add(d[:], q3[:, :, 0], q3[:, :, 1])
    V.tensor_add(d[:], d[:], q3[:, :, 2])
    dd = ptile((B, J), "rg_dd")
    V.tensor_scalar(dd[:], d[:], -1.0, 1.0, ALU.mult, ALU.add)
    snv = ptile((B, 3 * J), "rg_snv")
    s3 = snv[:].rearrange("p (j k) -> p j k", k=3)
    V.tensor_mul(s3, n3, sn[:].unsqueeze(2).broadcast_to([B, J, 3]))
    r4 = rot[:].rearrange("p (j m n) -> p j m n", m=3, n=3)
    for m in range(3):
        V.tensor_add(r4[:, :, m, m], q3[:, :, m], dd[:])
    p = ptile((B, J), "rg_p")
    V.tensor_mul(p[:], un3[:, :, 0], n3[:, :, 1])
    V.tensor_sub(r4[:, :, 0, 1], p[:], s3[:, :, 2])
    V.tensor_add(r4[:, :, 1, 0], p[:], s3[:, :, 2])
    V.tensor_mul(p[:], un3[:, :, 0], n3[:, :, 2])
    V.tensor_add(r4[:, :, 0, 2], p[:], s3[:, :, 1])
    V.tensor_sub(r4[:, :, 2, 0], p[:], s3[:, :, 1])
    V.tensor_mul(p[:], un3[:, :, 1], n3[:, :, 2])
    V.tensor_sub(r4[:, :, 1, 2], p[:], s3[:, :, 0])
    V.tensor_add(r4[:, :, 2, 1], p[:], s3[:, :, 0])


# ================================================================ entry

_CACHED = {}
DEBUG = False


def _get_nc():
    if "nc" not in _CACHED:
        _CACHED["nc"] = _build_nc()
    return _CACHED["nc"]


PROFILE = False


def kernel(**inputs):
    in_maps, vid_all = _host_prep(inputs)
    nc = _get_nc()
    res = run_bass_kernel_spmd(nc, in_maps, core_ids=list(range(NCORES)),
                               trace=PROFILE)
    _CACHED["last_res"] = res
    out = np.zeros((B, VS, 3), np.float32)
    for c in range(NCORES):
        o = np.asarray(res.results[c]["out"]).astype(np.float32).reshape(ROWS, 3, B)
        vok = vid_all[c] >= 0
        out[:, vid_all[c][vok], :] = o[vok].transpose(2, 0, 1)
    return out


# revision 41
# speedup vs baseline: 1.0218x; 1.0218x over previous
"""EHM (SMPLX body + FLAME head + MANO hands) Bass kernel for 8 TRN2 NeuronCores.

Sharding: VERTEX sharding -- model weights (shapedirs/posedirs/lbs weights)
dominate HBM traffic, so each core owns 1/8 of the SMPLX vertices (plus the
FLAME/MANO vertices its SMPLX rows stitch in) and computes ALL B=128 batch
elements for its shard.

v2 key restructure vs v1: joint regression is linear in betas, so
J = J_reg @ (template + shapedirs @ beta) is host-precomputed as
Jdirs = J_reg @ [shapedirs | template]  (tiny: 55x3x351).  On device the
joints come from a small Jdirs @ betaT matmul -- NO AllReduce, NO dependency
of FK on the big blend-shape stage.  FK (replicated, batch-on-partitions,
vector engine) fully overlaps the shapedirs/posedirs matmul stage.
Consequences: FLAME "even" chunks, MANO J chunks, smplx J regressor slabs all
vanish; head/hand chunks don't need smplx shapedirs (their rows get
overwritten by stitching); MANO rest joints are fully host-computed (betas
are batch-constant).

Per-vertex data layout: [vertex(partition<=128), (c, b)] with c-major free dim
(col = c*128 + b).  Batch-staged data (poses, FK, A matrices): [b(part), free].
"""

import sys

sys.path.insert(0, "/opt/trn_rl_repo")

from contextlib import ExitStack

import numpy as np
import ml_dtypes

BF16NP = ml_dtypes.bfloat16

import concourse.bass as bass
import concourse.bacc as bacc
import concourse.tile as tile
import concourse.mybir as mybir
from concourse.bass_utils import run_bass_kernel_spmd

F32 = mybir.dt.float32
BF16 = mybir.dt.bfloat16
AF = mybir.ActivationFunctionType
ALU = mybir.AluOpType

# ---------------------------------------------------------------- constants
B = 128
VS, VF, VM = 10475, 5023, 778
NL = 350
NCORES = 8

SMPLX_PARENTS = np.array([-1,0,0,0,1,2,3,4,5,6,7,8,9,9,9,12,13,14,16,17,18,19,
                          15,15,15,20,25,26,20,28,29,20,31,32,20,34,35,20,37,38,
                          21,40,41,21,43,44,21,46,47,21,49,50,21,52,53])
FLAME_PARENTS = np.array([-1,0,1,1,1])
MANO_PARENTS = np.array([-1,0,1,2,0,4,5,0,7,8,0,10,11,0,13,14])

N_PLAIN, N_HEAD, N_HL, N_HR = 768, 384, 128, 128
ROWS = N_PLAIN + N_HEAD + N_HL + N_HR        # 1408
NCH = ROWS // 128                            # 11
NCH_PLAIN = 6
CH_HEAD0 = 6                                 # chunks 6,7,8 head; 9 L; 10 R
CH_HL, CH_HR = 9, 10

PD_S_K = 189
PD_F_K = 27
PD_M_K = 135

NJ_ALL = 92
OFF_S, OFF_F, OFF_L, OFF_R = 0, 55, 60, 76
NROT = 55
ROT_S0, ROT_F0, ROT_L0, ROT_R0 = 0, 22, 25, 40

BF16_INPUTS = {"w_s", "wre_f", "w_m",
               "sd_s", "pd_s_a", "pd_s_b", "sd_f", "pd_f",
               "sd_m", "pd_m_a", "pd_m_b",
               "betaT_s", "betaT_f", "betam_rep", "jd_s", "jd_f"}


def _fk_forest():
    par = np.empty(NJ_ALL, np.int64)
    par[OFF_S:OFF_S + 55] = SMPLX_PARENTS
    par[OFF_F:OFF_F + 5] = np.where(FLAME_PARENTS < 0, -1, FLAME_PARENTS + OFF_F)
    par[OFF_L:OFF_L + 16] = np.where(MANO_PARENTS < 0, -1, MANO_PARENTS + OFF_L)
    par[OFF_R:OFF_R + 16] = np.where(MANO_PARENTS < 0, -1, MANO_PARENTS + OFF_R)
    return par


def _fk_levels(par):
    depth = np.zeros(NJ_ALL, np.int64)
    for j in range(NJ_ALL):
        if par[j] >= 0:
            depth[j] = depth[par[j]] + 1
    levels = []
    for d in range(1, int(depth.max()) + 1):
        js = np.nonzero(depth == d)[0]
        runs, i = [], 0
        while i < len(js):
            j0, p0 = int(js[i]), int(par[js[i]])
            if i + 1 < len(js):
                ds = int(js[i + 1]) - j0
                ps = int(par[js[i + 1]]) - p0
            else:
                ds, ps = 1, 0
            n = 1
            while (i + n < len(js) and int(js[i + n]) == j0 + n * ds
                   and int(par[js[i + n]]) == p0 + n * ps):
                n += 1
            if n == 1:
                ds, ps = 1, 0
            runs.append((j0, ds, n, p0, ps))
            i += n
        levels.append(runs)
    return levels


def _fk_levels_split():
    """Split forest levels into smplx-only runs and flame/mano runs (the trees
    are disjoint, so the smplx chain can be processed first)."""
    levels = _fk_levels(_fk_forest())
    ls, lfm = [], []
    for runs in levels:
        rs = [r for r in runs if r[0] < 55]
        rf = [r for r in runs if r[0] >= 55]
        if rs: ls.append(rs)
        if rf: lfm.append(rf)
    return ls, lfm


# ================================================================ host prep

def _split_sizes(total, parts):
    q, r = divmod(total, parts)
    return [q + (1 if i < r else 0) for i in range(parts)]


def _pad_ids(ids, n):
    out = np.full(n, -1, np.int64)
    out[:len(ids)] = ids
    return out


def _host_prep(inp):
    f32 = np.float32
    s2f = np.asarray(inp["smplx2flame_ind"])
    head_ix = np.asarray(inp["head_index"])
    s2l = np.asarray(inp["smplx2mano_left"])
    s2r = np.asarray(inp["smplx2mano_right"])

    head_sv = s2f[head_ix]
    special = np.zeros(VS, bool)
    special[head_sv] = True
    special[s2l] = True
    special[s2r] = True
    plain_sv = np.nonzero(~special)[0]

    pl_sp = np.cumsum([0] + _split_sizes(len(plain_sv), NCORES))
    hd_sp = np.cumsum([0] + _split_sizes(len(head_ix), NCORES))
    hl_sp = np.cumsum([0] + _split_sizes(VM, NCORES))

    sd_s_np = np.asarray(inp["smplx_shapedirs"], f32)
    pd_s_np = np.asarray(inp["smplx_posedirs"], f32)
    jr_s_np = np.asarray(inp["smplx_J_regressor"], f32)
    w_s_np = np.asarray(inp["smplx_lbs_weights"], f32)
    tmpl_s = np.asarray(inp["smplx_v_template"], f32)
    sd_f_np = np.asarray(inp["flame_shapedirs"], f32)
    pd_f_np = np.asarray(inp["flame_posedirs"], f32)
    jr_f_np = np.asarray(inp["flame_J_regressor"], f32)
    w_f_np = np.asarray(inp["flame_lbs_weights"], f32)
    tmpl_f = np.asarray(inp["flame_v_template"], f32)
    re_np = np.asarray(inp["r_eyelid"], f32)
    le_np = np.asarray(inp["l_eyelid"], f32)
    sd_m_np = np.asarray(inp["mano_shapedirs"], f32)
    pd_m_np = np.asarray(inp["mano_posedirs"], f32)
    jr_m_np = np.asarray(inp["mano_J_regressor"], f32)
    w_m_np = np.asarray(inp["mano_lbs_weights"], f32)
    tmpl_m = np.asarray(inp["mano_v_template"], f32)

    aa = np.concatenate([
        np.asarray(inp["global_pose"], f32).reshape(B, 3),
        np.asarray(inp["body_pose"], f32).reshape(B, 63),
        np.asarray(inp["jaw_params"], f32).reshape(B, 3),
        np.asarray(inp["eye_pose"], f32).reshape(B, 6),
        np.asarray(inp["left_hand_pose"], f32).reshape(B, 45),
        np.asarray(inp["right_hand_pose"], f32).reshape(B, 45),
    ], axis=1)

    # ---- MANO rest joints: batch-constant -> fully host-computed ----------
    betam_vec = np.asarray(inp["mano_betas"], f32)[0]           # [10]
    vshaped_m = tmpl_m + sd_m_np @ betam_vec                    # [VM, 3]
    J_m = jr_m_np @ vshaped_m                                   # [16, 3]
    rel_m = J_m.copy()
    rel_m[1:] -= J_m[MANO_PARENTS[1:]]
    jmb_rep = np.tile(np.ascontiguousarray(J_m.T).reshape(-1), (B, 1))
    relmb_rep = np.tile(np.ascontiguousarray(rel_m.T).reshape(-1), (B, 1))

    ep = np.asarray(inp["eyelid_params"], f32)
    aux = np.concatenate([
        np.asarray(inp["head_scale"], f32)[:, None],
        np.asarray(inp["left_hand_scale"], f32)[:, None],
        np.asarray(inp["right_hand_scale"], f32)[:, None],
        ep[:, 0:1], ep[:, 1:2],
        np.asarray(inp["head_pos_offset"], f32),
        np.asarray(inp["left_hand_pos_offset"], f32) - J_m[0][None],
        np.asarray(inp["right_hand_pos_offset"], f32) - J_m[0][None],
    ], axis=1)                                               # [128, 14]

    def beta_T(second):
        b = np.concatenate([np.asarray(inp["shape_params"], f32), second], 1)
        bt = np.zeros((384, B), f32)
        bt[:NL] = b.T
        bt[NL] = 1.0
        return bt.reshape(3, 128, B)

    betaT_s = beta_T(np.asarray(inp["body_exp"], f32))
    betaT_f = beta_T(np.asarray(inp["flame_exp"], f32))

    joff = np.asarray(inp["joints_offset"], f32)
    joffT = np.ascontiguousarray(joff.transpose(1, 2, 0)).reshape(55, 384)

    # ---- precomputed joint regressor directions: J = jd . [beta;1] --------
    def jdirs(jr, sd, tmpl, nj):
        ja = (jr @ sd.reshape(-1, 3 * NL)).reshape(nj, 3, NL)   # [nj,3,350]
        jt = jr @ tmpl                                          # [nj,3]
        out = np.zeros((3, 3, 128, nj), f32)
        for c in range(3):
            full = np.zeros((384, nj), f32)
            full[:NL] = ja[:, c, :].T
            full[NL] = jt[:, c]
            out[c] = full.reshape(3, 128, nj)
        return out.reshape(9, 128, nj)

    jd_s = jdirs(jr_s_np, sd_s_np, tmpl_s, 55)
    jd_f = jdirs(jr_f_np, sd_f_np, tmpl_f, 5)

    def mrel_T(par, nj):
        m = np.eye(nj, dtype=f32)
        for j in range(1, nj):
            if par[j] >= 0:
                m[j, par[j]] = -1.0
        return np.ascontiguousarray(m.T)

    betam_rep = np.zeros((11, 128), f32)
    betam_rep[:10] = betam_vec[:, None]
    betam_rep[10] = 1.0

    bpack = np.concatenate([aa, aux, jmb_rep, relmb_rep], 1)       # [128, 275]
    spack = np.zeros((55, 444), f32)
    spack[:, 0:55] = mrel_T(SMPLX_PARENTS, 55)
    spack[0:5, 55:60] = mrel_T(FLAME_PARENTS, 5)
    spack[:, 60:444] = joffT

    rep = dict(bpack=bpack, spack=spack, betaT_s=betaT_s, betaT_f=betaT_f,
               betam_rep=betam_rep, jd_s=jd_s, jd_f=jd_f,
               ident=np.eye(128, dtype=f32))

    in_maps = []
    vid_all = np.full((NCORES, ROWS), -1, np.int64)

    for c in range(NCORES):
        p_ids = plain_sv[pl_sp[c]:pl_sp[c + 1]]
        h_pos = np.arange(hd_sp[c], hd_sp[c + 1])
        h_sv, h_fv = head_sv[h_pos], head_ix[h_pos]
        l_pos = np.arange(hl_sp[c], hl_sp[c + 1])
        r_pos = l_pos                                         # same split for R
        l_sv, r_sv = s2l[l_pos], s2r[r_pos]

        vid = np.full(ROWS, -1, np.int64)
        vid[:len(p_ids)] = p_ids
        vid[N_PLAIN:N_PLAIN + len(h_sv)] = h_sv
        vid[N_PLAIN + N_HEAD:N_PLAIN + N_HEAD + len(l_sv)] = l_sv
        vid[N_PLAIN + N_HEAD + N_HL:N_PLAIN + N_HEAD + N_HL + len(r_sv)] = r_sv
        vid_all[c] = vid
        vok = vid >= 0
        vc = np.where(vok, vid, 0)

        # smplx shapedirs slab for PLAIN chunks only: [6, 128(p=l), (c, lk, v)]
        pv, pok = vc[:N_PLAIN], vok[:N_PLAIN]
        sdp = np.zeros((N_PLAIN, 3, 384), f32)
        sdp[:, :, :NL] = np.where(pok[:, None, None], sd_s_np[pv], 0.0)
        sdp[:, :, NL] = np.where(pok[:, None], tmpl_s[pv], 0.0)
        slab = sdp.reshape(NCH_PLAIN, 128, 3, 3, 128).transpose(0, 4, 2, 3, 1)
        sd_s = np.ascontiguousarray(slab).reshape(NCH_PLAIN, 128, 1152)

        colv = vc[:, None] * 3 + np.arange(3)[None, :]
        pdv = pd_s_np[:PD_S_K][:, colv]
        pdv = np.where(vok[None, :, None], pdv, 0.0)
        pdv = pdv.reshape(PD_S_K, NCH, 128, 3).transpose(1, 0, 3, 2)
        pd_s_a = np.ascontiguousarray(pdv[:, :128]).reshape(NCH, 128, 384)
        pd_s_b = np.ascontiguousarray(pdv[:, 128:]).reshape(NCH, PD_S_K - 128, 384)

        w_s = np.ascontiguousarray(
            np.where(vok[:, None], w_s_np[vc], 0.0)
            .reshape(NCH, 128, 55).transpose(0, 2, 1))

        # flame: 3 gathered chunks (only vertices actually stitched)
        fg = _pad_ids(h_fv, N_HEAD)
        fok = fg >= 0
        fc = np.where(fok, fg, 0)
        sdfp = np.zeros((N_HEAD, 3, 384), f32)
        sdfp[:, :, :NL] = np.where(fok[:, None, None], sd_f_np[fc], 0.0)
        sdfp[:, :, NL] = np.where(fok[:, None], tmpl_f[fc], 0.0)
        slab = sdfp.reshape(-1, 128, 3, 3, 128).transpose(0, 4, 2, 3, 1)
        sd_f = np.ascontiguousarray(slab).reshape(-1, 128, 1152)

        colf = fc[:, None] * 3 + np.arange(3)[None, :]
        pdfv = pd_f_np[9:36][:, colf]
        pdfv = np.where(fok[None, :, None], pdfv, 0.0)
        pdfv = pdfv.reshape(PD_F_K, 3, 128, 3).transpose(1, 0, 3, 2)
        pd_f = np.ascontiguousarray(pdfv).reshape(3, PD_F_K, 384)

        wre = np.zeros((3, 12, 128), f32)
        for k in range(3):
            rows, ok = fc[k * 128:(k + 1) * 128], fok[k * 128:(k + 1) * 128]
            wre[k, :5] = np.where(ok[None, :], w_f_np[rows].T, 0.0)
            wre[k, 5:8] = np.where(ok[None, :], re_np[rows].T, 0.0)
            wre[k, 8:11] = np.where(ok[None, :], le_np[rows].T, 0.0)
            wre[k, 11] = 1.0                                  # bias row

        # mano hands
        m_rows = np.stack([_pad_ids(l_pos, 128), _pad_ids(r_pos, 128)])
        mok = m_rows >= 0
        mc = np.where(mok, m_rows, 0)
        sd_m = np.zeros((2, 11, 384), f32)
        pd_m_a = np.zeros((2, 128, 384), f32)
        pd_m_b = np.zeros((2, PD_M_K - 128, 384), f32)
        w_m = np.zeros((2, 34, 128), f32)
        for h in range(2):
            sdm = np.where(mok[h][:, None, None], sd_m_np[mc[h]], 0.0)
            sd_m[h, :10] = sdm.transpose(2, 1, 0).reshape(10, 384)
            sd_m[h, 10] = np.where(mok[h][:, None], tmpl_m[mc[h]], 0.0).T.reshape(384)
            colm = mc[h][:, None] * 3 + np.arange(3)[None, :]
            pdm = pd_m_np[:, colm]
            pdm = np.where(mok[h][None, :, None], pdm, 0.0).transpose(0, 2, 1)
            pd_m_a[h] = pdm[:128].reshape(128, 384)
            pd_m_b[h] = pdm[128:].reshape(PD_M_K - 128, 384)
            w_m[h, h * 16:h * 16 + 16] = np.where(mok[h][None, :],
                                                  w_m_np[mc[h]].T, 0.0)
            w_m[h, 32 + h] = 1.0                              # bias row

        m = dict(rep)
        m.update(sd_s=sd_s, pd_s_a=pd_s_a, pd_s_b=pd_s_b, w_s=w_s,
                 sd_f=sd_f, pd_f=pd_f, wre_f=wre,
                 sd_m=sd_m, pd_m_a=pd_m_a, pd_m_b=pd_m_b, w_m=w_m)
        out = {}
        for k, v in m.items():
            if k in BF16_INPUTS:
                out[k] = np.ascontiguousarray(v.astype(BF16NP))
            else:
                out[k] = np.ascontiguousarray(v, f32)
        in_maps.append(out)

    return in_maps, vid_all


# ================================================================ device IR

def _build_nc():
    nc = bacc.Bacc("TRN2", target_bir_lowering=False, debug=False,
                   num_devices=NCORES)
    di = {}

    def din(name, shape):
        dt = BF16 if name in BF16_INPUTS else F32
        di[name] = nc.dram_tensor(name, list(shape), dt, kind="ExternalInput").ap()

    din("bpack", (B, 275)); din("spack", (55, 444))
    din("betaT_s", (3, 128, 128)); din("betaT_f", (3, 128, 128))
    din("betam_rep", (11, 128)); din("ident", (128, 128))
    din("jd_s", (9, 128, 55)); din("jd_f", (9, 128, 5))
    din("sd_s", (NCH_PLAIN, 128, 1152))
    din("pd_s_a", (NCH, 128, 384)); din("pd_s_b", (NCH, PD_S_K - 128, 384))
    din("w_s", (NCH, 55, 128))
    din("sd_f", (3, 128, 1152))
    din("pd_f", (3, PD_F_K, 384)); din("wre_f", (3, 12, 128))
    din("sd_m", (2, 11, 384)); din("pd_m_a", (2, 128, 384))
    din("pd_m_b", (2, PD_M_K - 128, 384)); din("w_m", (2, 34, 128))

    out_d = nc.dram_tensor("out", [ROWS, 384], BF16, kind="ExternalOutput").ap()
    dbg_d = None
    if DEBUG:
        dbg_d = nc.dram_tensor("dbg", [128, 8192], F32, kind="ExternalOutput").ap()

    with tile.TileContext(nc) as tc:
        _emit(nc, tc, di, out_d, dbg_d)
    nc.compile()
    return nc


def _emit(nc, tc, di, out_d, dbg_d=None):
    levels_s, levels_fm = _fk_levels_split()
    es = ExitStack()
    persist = es.enter_context(tc.tile_pool(name="persist", bufs=1))
    slabs = es.enter_context(tc.tile_pool(name="slabs", bufs=3))
    acc_cm = tc.tile_pool(name="acc", bufs=2, space="PSUM")
    acc = acc_cm.__enter__()
    big_cm = tc.tile_pool(name="big", bufs=2, space="PSUM")
    big = big_cm.__enter__()

    V, S, G, T, DMA = nc.vector, nc.scalar, nc.gpsimd, nc.tensor, nc.sync

    def ptile(shape, name):
        return persist.tile(list(shape), F32, tag=name, name=name)

    def btile(shape, name):
        return persist.tile(list(shape), BF16, tag=name, name=name)

    # ---------------- staged inputs ---------------------------------------
    # Each dma_start costs ~1.4us of issue time on its queue, so inputs are
    # packed into few transfers and spread over the three DGE queues
    # (Sync / Scalar / GpSimd) with the critical path (aa -> rodrigues,
    # jds/betaT -> joints -> FK) first on Sync.
    bpack = ptile((B, 275), "bpack")
    G.dma_start(bpack[:], di["bpack"][:])
    aa = bpack[:, 0:165]
    aux = bpack[:, 165:179]
    jmb = bpack[:, 179:227]
    relmb = bpack[:, 227:275]
    jds = btile((128, 9 * 55), "jds")
    DMA.dma_start(jds[:].rearrange("p (k j) -> p k j", j=55),
                  di["jd_s"][:].rearrange("k p j -> p k j"))
    betaT_s = btile((128, 384), "betaT_s")
    DMA.dma_start(betaT_s[:].rearrange("p (k b) -> p k b", b=128),
                  di["betaT_s"][:].rearrange("k p b -> p k b"))
    ident = ptile((128, 128), "ident")
    DMA.dma_start(ident[:], di["ident"][:])
    # flame-path inputs off the Sync queue (flame J is not start-critical)
    jdf = btile((128, 9 * 5), "jdf")
    S.dma_start(jdf[:].rearrange("p (k j) -> p k j", j=5),
                di["jd_f"][:].rearrange("k p j -> p k j"))
    betaT_f = btile((128, 384), "betaT_f")
    S.dma_start(betaT_f[:].rearrange("p (k b) -> p k b", b=128),
                di["betaT_f"][:].rearrange("k p b -> p k b"))
    spack = ptile((55, 444), "spack")
    G.dma_start(spack[:], di["spack"][:])
    mrelT_s = spack[:, 0:55]
    mrelT_f = spack[0:5, 55:60]
    joffT = spack[:, 60:444]
    betam = btile((11, 128), "betam")
    G.dma_start(betam[:], di["betam_rep"][:])

    # ---------------- rodrigues (V + S) -----------------------------------
    rot = ptile((B, NROT * 9), "rot")
    _rodrigues(nc, aa, rot, ptile)
    rot4 = rot[:].rearrange("p (j x) -> p j x", x=9)

    # skinning weights preloaded via the Scalar DGE queue (after rodrigues
    # so its activations are not delayed)
    w_all = persist.tile([55, NCH * 128], BF16, tag="w_all", name="w_all")
    S.dma_start(w_all[:].rearrange("j (i b) -> j i b", b=128),
                di["w_s"][:].rearrange("i j b -> j i b"))
    wre_all = persist.tile([12, 384], BF16, tag="wre_all", name="wre_all")
    S.dma_start(wre_all[:].rearrange("r (h b) -> r h b", b=128),
                di["wre_f"][:].rearrange("h r b -> r h b"))
    wm_all = persist.tile([34, 256], BF16, tag="wm_all", name="wm_all")
    S.dma_start(wm_all[:].rearrange("r (h b) -> r h b", b=128),
                di["w_m"][:].rearrange("h r b -> r h b"))

    # ---------------- joints from betas (tensor, tiny) ---------------------
    jp = acc.tile([128, 384], F32, tag="acc", padded_shape=[128, 512])
    for c3 in range(3):
        for lk in range(3):
            T.matmul(jp[0:55, c3 * 128:(c3 + 1) * 128],
                     jds[:, (c3 * 3 + lk) * 55:(c3 * 3 + lk + 1) * 55],
                     betaT_s[:, lk * 128:(lk + 1) * 128],
                     start=(lk == 0), stop=(lk == 2))
    tbj = ptile((55, 384), "tbj")
    V.tensor_add(tbj[:], jp[0:55, :], joffT)

    jpf = acc.tile([128, 384], F32, tag="acc", padded_shape=[128, 512])
    for c3 in range(3):
        for lk in range(3):
            T.matmul(jpf[0:5, c3 * 128:(c3 + 1) * 128],
                     jdf[:, (c3 * 3 + lk) * 5:(c3 * 3 + lk + 1) * 5],
                     betaT_f[:, lk * 128:(lk + 1) * 128],
                     start=(lk == 0), stop=(lk == 2))
    arr_f = ptile((5, 384), "arr_f")
    S.copy(arr_f[:], jpf[0:5, :])

    # rel joints
    rel_s = ptile((55, 384), "rel_s")
    pp = acc.tile([128, 384], F32, tag="acc", padded_shape=[128, 512])
    T.matmul(pp[0:55, :], mrelT_s, tbj[:], start=True, stop=True)
    S.copy(rel_s[:], pp[0:55, :])
    rel_f = ptile((5, 384), "rel_f")
    pp = acc.tile([128, 384], F32, tag="acc", padded_shape=[128, 512])
    T.matmul(pp[0:5, :], mrelT_f, arr_f[:], start=True, stop=True)
    S.copy(rel_f[:], pp[0:5, :])

    def transpose_to(dst_ap, src_ap):
        pq = acc.tile([128, 384], F32, tag="acc", padded_shape=[128, 512])
        k, n = src_ap.shape[0], src_ap.shape[1]
        T.matmul(pq[:n, :k], src_ap, ident[:k, :k], is_transpose=True,
                 start=True, stop=True)
        S.copy(dst_ap, pq[:n, :k])

    # batch-major staging of joints / rel for FK
    jb = ptile((B, 165), "jb")
    relb = ptile((B, 165), "relb")
    jfb = ptile((B, 15), "jfb")
    relfb = ptile((B, 15), "relfb")
    for c3 in range(3):
        transpose_to(jb[:, c3 * 55:(c3 + 1) * 55], tbj[:, c3 * 128:(c3 + 1) * 128])
        transpose_to(relb[:, c3 * 55:(c3 + 1) * 55], rel_s[:, c3 * 128:(c3 + 1) * 128])
        transpose_to(jfb[:, c3 * 5:(c3 + 1) * 5], arr_f[:, c3 * 128:(c3 + 1) * 128])
        transpose_to(relfb[:, c3 * 5:(c3 + 1) * 5], rel_f[:, c3 * 128:(c3 + 1) * 128])

    # pf = rot - I staged for posedirs matmuls (transposed, bf16)
    def pf_make(name, j0, n):
        t = ptile((B, n * 9), name)
        t9 = t[:].rearrange("p (j x) -> p j x", x=9)
        V.tensor_copy(t9, rot4[:, j0:j0 + n, :])
        V.tensor_scalar_add(t9[:, :, 0:9:4], t9[:, :, 0:9:4], -1.0)
        return t

    pf_s = pf_make("pf_s", 1, 21)
    pf_f = pf_make("pf_f", 22, 3)
    pf_m = [pf_make("pf_l", 25, 15), pf_make("pf_r", 40, 15)]

    pfT_s_a = btile((128, 128), "pfT_s_a")
    pfT_s_b = btile((PD_S_K - 128, 128), "pfT_s_b")
    transpose_to(pfT_s_a[:], pf_s[:, 0:128])
    transpose_to(pfT_s_b[:], pf_s[:, 128:PD_S_K])
    pfT_f = btile((PD_F_K, 128), "pfT_f")
    transpose_to(pfT_f[:], pf_f[:, :])
    pfT_m_a = [btile((128, 128), "pfT_l_a"), btile((128, 128), "pfT_r_a")]
    pfT_m_b = [btile((PD_M_K - 128, 128), "pfT_l_b"),
               btile((PD_M_K - 128, 128), "pfT_r_b")]
    for h in range(2):
        transpose_to(pfT_m_a[h][:], pf_m[h][:, 0:128])
        transpose_to(pfT_m_b[h][:], pf_m[h][:, 128:PD_M_K])

    # ---------------- FK (vector, batch on partitions) ---------------------
    Tb = ptile((B, NJ_ALL * 12), "Tb")
    Ab = ptile((B, NJ_ALL * 12), "Ab")
    T4 = Tb[:].rearrange("p (j m n) -> p j m n", m=3, n=4)
    A4 = Ab[:].rearrange("p (j m n) -> p j m n", m=3, n=4)
    G.memset(Tb[:], 0.0)
    for j0, n in ((22, 33), (OFF_F, 2), (OFF_L, 1), (OFF_R, 1)):
        G.memset(Tb[:].rearrange("p (j x) -> p j x", x=12)[:, j0:j0 + n, 0:11:5], 1.0)
    # rhs memsets done early while GpSimd is idle
    rhs_f = persist.tile([12, 1536], BF16, tag="rhs_f", name="rhs_f")
    G.memset(rhs_f[:], 0.0)
    rhs_lr = persist.tile([34, 1536], BF16, tag="rhs_lr", name="rhs_lr")
    G.memset(rhs_lr[:], 0.0)

    def rot_to_T(tj0, rj0, n):
        V.tensor_copy(T4[:, tj0:tj0 + n, :, 0:3],
                      rot4[:, rj0:rj0 + n, :].rearrange("p j (m n) -> p j m n", n=3))

    def fk_run(runs_list):
        fk_tmp2 = ptile((B, 12 * 16), "fk_tmp2")
        for runs in runs_list:
            for (d0, ds, n, p0, ps) in runs:
                sl_d = slice(d0, d0 + (n - 1) * ds + 1, ds) if ds != 1 else slice(d0, d0 + n)
                dst, dT = A4[:, sl_d], T4[:, sl_d]
                if ps == 0:
                    par = A4[:, p0:p0 + 1].broadcast_to([B, n, 3, 4])
                else:
                    sl_p = slice(p0, p0 + (n - 1) * ps + 1, ps) if ps != 1 else slice(p0, p0 + n)
                    par = A4[:, sl_p]
                sc2 = fk_tmp2[:].rearrange("p (j m n) -> p j m n", m=3, n=4)[:, :n]
                for k in range(3):
                    a_k = par[:, :, :, k:k + 1].broadcast_to([B, n, 3, 4])
                    t_k = dT[:, :, k:k + 1, :].broadcast_to([B, n, 3, 4])
                    if k == 0:
                        V.tensor_mul(dst, a_k, t_k)
                    else:
                        V.tensor_mul(sc2, a_k, t_k)
                        V.tensor_add(dst, dst, sc2)
                V.tensor_add(dst[:, :, :, 3], dst[:, :, :, 3], par[:, :, :, 3])

    corr_tmp = ptile((B, NJ_ALL * 3), "corr_tmp")
    corr_tmp2 = ptile((B, NJ_ALL * 3), "corr_tmp2")

    def corr(j0, nj, jsrc):
        ct = corr_tmp[:].rearrange("p (j m) -> p j m", m=3)[:, 0:nj]
        ct2 = corr_tmp2[:].rearrange("p (j m) -> p j m", m=3)[:, 0:nj]
        js = jsrc.rearrange("p (c j) -> p c j", c=3)
        for k in range(3):
            a_k = A4[:, j0:j0 + nj, :, k]
            j_k = js[:, k, :].unsqueeze(2).broadcast_to([B, nj, 3])
            if k == 0:
                V.tensor_mul(ct, a_k, j_k)
            else:
                V.tensor_mul(ct2, a_k, j_k)
                V.tensor_add(ct, ct, ct2)
        V.tensor_sub(A4[:, j0:j0 + nj, :, 3], A4[:, j0:j0 + nj, :, 3], ct)

    # ---- smplx chain first: fills, levels, corr --------------------------
    rot_to_T(0, ROT_S0, 22)
    V.tensor_copy(T4[:, 0:55, :, 3], relb[:].rearrange("p (c j) -> p j c", c=3))
    V.tensor_copy(A4[:, 0:1], T4[:, 0:1])
    fk_run(levels_s)
    corr(OFF_S, 55, jb[:])

    # ================= stage A part 1: plain + hand chunks =================
    vp_sbuf = [btile((128, 384), f"vp{i}") for i in range(NCH)]
    vpf_sbuf = [btile((128, 384), f"vpf{h}") for h in range(3)]
    vpm_sbuf = [btile((128, 384), f"vpm{h}") for h in range(2)]

    def copy_vp(dst_t, pq3):
        S.copy(dst_t[:, 0:384].rearrange("p (c b) -> p c b", b=128),
               pq3[:].rearrange("p (c x) -> p c x", x=512)[:, :, 0:128])

    def stage_a_chunk(i, pda=None, pdb=None):
        pq3 = big.tile([128, 1536], F32, tag="bigp")
        sdt = None
        if i < NCH_PLAIN:
            sdt = slabs.tile((128, 1152), BF16, tag="sd_s")
            DMA.dma_start(sdt[:], di["sd_s"][i])
        if pda is None:
            pda = slabs.tile((128, 384), BF16, tag="pd_s_a")
            pdb = slabs.tile((PD_S_K - 128, 384), BF16, tag="pd_s_b")
            DMA.dma_start(pda[:], di["pd_s_a"][i])
            DMA.dma_start(pdb[:], di["pd_s_b"][i])
        for c3 in range(3):
            r = slice(c3 * 512, c3 * 512 + 128)
            if sdt is not None:
                for lk in range(3):
                    T.matmul(pq3[:, r],
                             sdt[:, (c3 * 3 + lk) * 128:(c3 * 3 + lk + 1) * 128],
                             betaT_s[:, lk * 128:(lk + 1) * 128],
                             start=(lk == 0), stop=False)
        for c3 in range(3):
            r = slice(c3 * 512, c3 * 512 + 128)
            T.matmul(pq3[:, r], pda[:, c3 * 128:(c3 + 1) * 128], pfT_s_a[:],
                     start=(sdt is None), stop=False)
        for c3 in range(3):
            r = slice(c3 * 512, c3 * 512 + 128)
            T.matmul(pq3[:, r], pdb[:, c3 * 128:(c3 + 1) * 128], pfT_s_b[:],
                     start=False, stop=True)
        if i in (CH_HL, CH_HR):
            h = i - CH_HL
            sdm = slabs.tile((11, 384), BF16, tag="sd_m")
            DMA.dma_start(sdm[:], di["sd_m"][h])
            pma = slabs.tile((128, 384), BF16, tag="pd_m_a")
            pmb = slabs.tile((PD_M_K - 128, 384), BF16, tag="pd_m_b")
            DMA.dma_start(pma[:], di["pd_m_a"][h])
            DMA.dma_start(pmb[:], di["pd_m_b"][h])
            pq2 = big.tile([128, 1536], F32, tag="bigp")
            for c3 in range(3):
                T.matmul(pq2[:, c3 * 512:c3 * 512 + 128],
                         sdm[:, c3 * 128:(c3 + 1) * 128], betam[:],
                         start=True, stop=False)
            for c3 in range(3):
                T.matmul(pq2[:, c3 * 512:c3 * 512 + 128],
                         pma[:, c3 * 128:(c3 + 1) * 128], pfT_m_a[h][:],
                         start=False, stop=False)
            for c3 in range(3):
                T.matmul(pq2[:, c3 * 512:c3 * 512 + 128],
                         pmb[:, c3 * 128:(c3 + 1) * 128], pfT_m_b[h][:],
                         start=False, stop=True)
            copy_vp(vpm_sbuf[h], pq2)
        copy_vp(vp_sbuf[i], pq3)

    # preload head/flame slabs via the (idle) GpSimd DGE queue so those
    # matmuls aren't gated by the backed-up Sync queue
    sdf_t, pdf_t, hpd_a, hpd_b = [], [], [], []
    for h in range(3):
        t = persist.tile([128, 1152], BF16, tag=f"sdf{h}", name=f"sdf{h}")
        G.dma_start(t[:], di["sd_f"][h]); sdf_t.append(t)
        t = persist.tile([PD_F_K, 384], BF16, tag=f"pdf{h}", name=f"pdf{h}")
        G.dma_start(t[:], di["pd_f"][h]); pdf_t.append(t)
        i = CH_HEAD0 + h
        t = persist.tile([128, 384], BF16, tag=f"hpa{h}", name=f"hpa{h}")
        G.dma_start(t[:], di["pd_s_a"][i]); hpd_a.append(t)
        t = persist.tile([PD_S_K - 128, 384], BF16, tag=f"hpb{h}", name=f"hpb{h}")
        G.dma_start(t[:], di["pd_s_b"][i]); hpd_b.append(t)

    for i in list(range(NCH_PLAIN)) + [CH_HL, CH_HR]:
        stage_a_chunk(i)
    for h in range(3):
        stage_a_chunk(CH_HEAD0 + h, hpd_a[h], hpd_b[h])

    # ---- rhs_s (only needs the smplx chain) -------------------------------
    def rhs_fill(rhs_t, j0, nj):
        bp = big.tile([128, 1536], F32, tag="bigp")
        for n4 in range(4):
            for m3 in range(3):
                T.matmul(bp[0:nj, n4 * 384 + m3 * 128:n4 * 384 + (m3 + 1) * 128],
                         A4[:, j0:j0 + nj, m3, n4], ident[:],
                         is_transpose=True, start=True, stop=True)
        S.copy(rhs_t[0:nj, 0:1536], bp[0:nj, :])

    rhs_s = persist.tile([55, 1536], BF16, tag="rhs_s", name="rhs_s")
    rhs_fill(rhs_s, 0, 55)

    # ---- flame/mano chains ------------------------------------------------
    rot_to_T(OFF_F + 2, ROT_F0, 3)
    rot_to_T(OFF_L + 1, ROT_L0, 15)
    rot_to_T(OFF_R + 1, ROT_R0, 15)
    V.tensor_copy(T4[:, OFF_F:OFF_F + 5, :, 3],
                  relfb[:].rearrange("p (c j) -> p j c", c=3))
    for off in (OFF_L, OFF_R):
        V.tensor_copy(T4[:, off:off + 16, :, 3],
                      relmb.rearrange("p (c j) -> p j c", c=3))
    for r in (OFF_F, OFF_L, OFF_R):
        V.tensor_copy(A4[:, r:r + 1], T4[:, r:r + 1])
    fk_run(levels_fm)

    # ---- per-batch staging (world translations BEFORE rel-correction) ----
    hm = ptile((B, 16), "hm")
    jb3 = jb[:].rearrange("p (c j) -> p c j", c=3)
    bias9 = ptile((B, 9), "bias9")
    V.tensor_add(hm[:, 0:3], jb3[:, :, 23], jb3[:, :, 24])
    V.tensor_add(hm[:, 3:6], A4[:, OFF_F + 3, :, 3], A4[:, OFF_F + 4, :, 3])
    V.tensor_sub(hm[:, 6:9], hm[:, 0:3], hm[:, 3:6])
    V.scalar_tensor_tensor(bias9[:, 0:3], hm[:, 6:9], 0.5, aux[:, 5:8],
                           ALU.mult, ALU.add)
    V.tensor_sub(bias9[:, 3:4], jb3[:, 0:1, 20], aux[:, 8:9])
    V.tensor_add(bias9[:, 4:6], aux[:, 9:11], jb3[:, 1:3, 20])
    V.tensor_add(bias9[:, 6:9], aux[:, 11:14], jb3[:, :, 21])
    epp = ptile((B, 2), "epp")
    V.tensor_mul(epp[:], aux[:, 3:5], aux[:, 0:1].broadcast_to([B, 2]))

    corr(OFF_F, 5, jfb[:])
    corr(OFF_L, 16, jmb)
    corr(OFF_R, 16, jmb)

    # ---- scale folding --------------------------------------------------
    V.tensor_scalar_mul(Ab[:, OFF_F * 12:(OFF_F + 5) * 12],
                        Ab[:, OFF_F * 12:(OFF_F + 5) * 12], aux[:, 0:1])
    negls = ptile((B, 1), "negls")
    V.tensor_scalar_mul(negls[:], aux[:, 1:2], -1.0)
    AL = A4[:, OFF_L:OFF_L + 16]
    V.tensor_scalar_mul(AL[:, :, 0, :], AL[:, :, 0, :], negls[:, 0:1])
    V.tensor_scalar_mul(AL[:, :, 1:3, :], AL[:, :, 1:3, :], aux[:, 1:2])
    ARr = A4[:, OFF_R:OFF_R + 16]
    V.tensor_scalar_mul(ARr[:, :, :, :], ARr[:, :, :, :], aux[:, 2:3])

    # ================= skinning: plain chunks ==============================
    scr_t = [btile((128, 384), f"scr{i}") for i in range(4)]
    gscr = [btile((128, 384), f"gscr{i}") for i in range(2)]

    def t_apply(E, dst_ap, tp_ap, x_t, scratch):
        """dst = sum_{n<3} T'[n]*x_n + T'[3]; layouts (n, m, b)."""
        d3 = dst_ap.rearrange("p (m b) -> p m b", b=128)
        x3 = x_t[:, 0:384].rearrange("p (c b) -> p c b", b=128)
        tp = tp_ap.rearrange("p (n m b) -> p n m b", m=3, b=128)
        sc = scratch.rearrange("p (m b) -> p m b", b=128)
        E.tensor_mul(d3, tp[:, 0], x3[:, 0:1].broadcast_to([128, 3, 128]))
        for n4 in (1, 2):
            E.tensor_mul(sc, tp[:, n4], x3[:, n4:n4 + 1].broadcast_to([128, 3, 128]))
            E.tensor_add(d3, d3, sc)
        E.tensor_add(d3, d3, tp[:, 3])

    def skin_chunk(i):
        tps = big.tile([128, 1536], F32, tag="bigp")
        for g in range(3):
            T.matmul(tps[:, g * 512:(g + 1) * 512],
                     w_all[:, i * 128:(i + 1) * 128],
                     rhs_s[:, g * 512:(g + 1) * 512], start=True, stop=True)
        tpb = slabs.tile((128, 1536), BF16, tag="tpb", bufs=3, name="tpb")
        S.copy(tpb[:], tps[:])
        ot = slabs.tile((128, 384), BF16, tag="outt", bufs=3, name="ot")
        t_apply(V, ot[:], tpb[:], vp_sbuf[i], scr_t[i % 4][:])
        DMA.dma_start(out_d[i * 128:(i + 1) * 128, :], ot[:])

    for i in range(NCH_PLAIN):
        skin_chunk(i)

    # ================= rhs_f / rhs_m =======================================
    epT = persist.tile([2, 128], BF16, tag="epT", name="epT")
    transpose_to(epT[:], epp[:, :])
    bias9T = persist.tile([9, 128], BF16, tag="bias9T", name="bias9T")
    transpose_to(bias9T[:], bias9[:, :])
    rhs_fill(rhs_lr, OFF_L, 32)
    rhs_fill(rhs_f, OFF_F, 5)
    for m3 in range(3):
        G.dma_start(rhs_f[5 + m3:6 + m3, (9 + m3) * 128:(10 + m3) * 128],
                    epT[1:2, :])
        G.dma_start(rhs_f[8 + m3:9 + m3, (9 + m3) * 128:(10 + m3) * 128],
                    epT[0:1, :])
        G.dma_start(rhs_f[11:12, (9 + m3) * 128:(10 + m3) * 128],
                    bias9T[m3:m3 + 1, :])
    for h in range(2):
        for m3 in range(3):
            G.dma_start(rhs_lr[32 + h:33 + h, (9 + m3) * 128:(10 + m3) * 128],
                        bias9T[3 + 3 * h + m3:4 + 3 * h + m3, :])

    # ================= skinning: head + hand chunks ========================
    # pre-skin (flame/mano LBS) offloaded to GpSimd from a Scalar-copied
    # SBUF image of the PSUM tile, overlapping the Vector final applies
    def pre_skin(i):
        tpx = big.tile([128, 1536], F32, tag="bigp")
        if i < CH_HEAD0 + 3 and i >= CH_HEAD0:
            h = i - CH_HEAD0
            wsl, rhs_x, x_t = wre_all[:, h * 128:(h + 1) * 128], rhs_f, vpf_sbuf[h]
        else:
            h = i - CH_HL
            wsl, rhs_x, x_t = wm_all[:, h * 128:(h + 1) * 128], rhs_lr, vpm_sbuf[h]
        for g in range(3):
            T.matmul(tpx[:, g * 512:(g + 1) * 512], wsl,
                     rhs_x[:, g * 512:(g + 1) * 512], start=True, stop=True)
        tpxb = slabs.tile((128, 1536), BF16, tag="tpb", bufs=3, name="tpb")
        S.copy(tpxb[:], tpx[:])
        hv = slabs.tile((128, 384), BF16, tag="hv", bufs=2, name="hv")
        t_apply(V, hv[:], tpxb[:], x_t, gscr[i % 2][:])
        G.tensor_add(vp_sbuf[i][:, 0:384], vp_sbuf[i][:, 0:384], hv[:])
        skin_chunk(i)

    for i in (CH_HL, CH_HR):
        pre_skin(i)

    # flame stage-A (only gates the head chunks; runs while hands finish)
    for h in range(3):
        pq3 = big.tile([128, 1536], F32, tag="bigp")
        sdt = sdf_t[h]
        pdf = pdf_t[h]
        for c3 in range(3):
            r = slice(c3 * 512, c3 * 512 + 128)
            for lk in range(3):
                T.matmul(pq3[:, r],
                         sdt[:, (c3 * 3 + lk) * 128:(c3 * 3 + lk + 1) * 128],
                         betaT_f[:, lk * 128:(lk + 1) * 128],
                         start=(lk == 0), stop=False)
        for c3 in range(3):
            r = slice(c3 * 512, c3 * 512 + 128)
            T.matmul(pq3[:, r], pdf[:, c3 * 128:(c3 + 1) * 128], pfT_f[:],
                     start=False, stop=True)
        copy_vp(vpf_sbuf[h], pq3)

    for i in range(CH_HEAD0, CH_HEAD0 + 3):
        pre_skin(i)

    if dbg_d is not None:
        DMA.dma_start(dbg_d[0:128, 0:495], rot[:])
        DMA.dma_start(dbg_d[0:128, 512:1616], Ab[:])
        DMA.dma_start(dbg_d[0:128, 1664:1829], jb[:])
        DMA.dma_start(dbg_d[0:128, 1856:2021], relb[:])
        DMA.dma_start(dbg_d[0:128, 3200:3209], bias9[:])
        DMA.dma_start(dbg_d[0:128, 3216:3232], hm[:])
        DMA.dma_start(dbg_d[0:55, 3712:4096], tbj[:])
        DMA.dma_start(dbg_d[0:55, 4096:4480], rel_s[:])
        DMA.dma_start(dbg_d[0:5, 4480:4864], arr_f[:])
        DMA.dma_start(dbg_d[0:5, 4864:5248], rel_f[:])
        DMA.dma_start(dbg_d[0:128, 5376:6480], Tb[:])
    big_cm.__exit__(None, None, None)
    acc_cm.__exit__(None, None, None)
    es.close()


def _rodrigues(nc, aa, rot, ptile):
    V, S = nc.vector, nc.scalar
    J = NROT
    aa3 = aa[:].rearrange("p (j k) -> p j k", k=3)
    sq = ptile((B, J), "rg_sq")
    tmp = ptile((B, J), "rg_tmp")
    V.tensor_mul(sq[:], aa3[:, :, 0], aa3[:, :, 0])
    V.tensor_mul(tmp[:], aa3[:, :, 1], aa3[:, :, 1])
    V.tensor_add(sq[:], sq[:], tmp[:])
    V.tensor_mul(tmp[:], aa3[:, :, 2], aa3[:, :, 2])
    V.tensor_add(sq[:], sq[:], tmp[:])
    eps_t = ptile((B, 1), "rg_eps")
    nc.gpsimd.memset(eps_t[:], 1e-8)
    hpi_t = ptile((B, 1), "rg_hpi")
    nc.gpsimd.memset(hpi_t[:], float(np.pi / 2))
    zero_t = ptile((B, 1), "rg_zero")
    nc.gpsimd.memset(zero_t[:], 0.0)
    ang = ptile((B, J), "rg_ang")
    S.activation(ang[:], sq[:], AF.Sqrt, bias=eps_t[:])
    inv = ptile((B, J), "rg_inv")
    V.reciprocal(inv[:], ang[:])
    sn = ptile((B, J), "rg_sin")
    co = ptile((B, J), "rg_cos")
    S.activation(sn[:], ang[:], AF.Sin, bias=zero_t[:])
    S.activation(co[:], ang[:], AF.Sin, bias=hpi_t[:])
    nv = ptile((B, 3 * J), "rg_n")
    n3 = nv[:].rearrange("p (j k) -> p j k", k=3)
    V.tensor_mul(n3, aa3, inv[:].unsqueeze(2).broadcast_to([B, J, 3]))
    u = ptile((B, J), "rg_u")
    V.tensor_scalar(u[:], co[:], -1.0, 1.0, ALU.mult, ALU.add)
    un = ptile((B, 3 * J), "rg_un")
    un3 = un[:].rearrange("p (j k) -> p j k", k=3)
    V.tensor_mul(un3, n3, u[:].unsqueeze(2).broadcast_to([B, J, 3]))
    q = ptile((B, 3 * J), "rg_q")
    q3 = q[:].rearrange("p (j k) -> p j k", k=3)
    V.tensor_mul(q3, un3, n3)
    d = ptile((B, J), "rg_d")
    V.tensor_add(d[:], q3[:, :, 0], q3[:, :, 1])
    V.tensor_add(d[:], d[:], q3[:, :, 2])
    dd = ptile((B, J), "rg_dd")
    V.tensor_scalar(dd[:], d[:], -1.0, 1.0, ALU.mult, ALU.add)
    snv = ptile((B, 3 * J), "rg_snv")
    s3 = snv[:].rearrange("p (j k) -> p j k", k=3)
    V.tensor_mul(s3, n3, sn[:].unsqueeze(2).broadcast_to([B, J, 3]))
    r4 = rot[:].rearrange("p (j m n) -> p j m n", m=3, n=3)
    for m in range(3):
        V.tensor_add(r4[:, :, m, m], q3[:, :, m], dd[:])
    p = ptile((B, J), "rg_p")
    V.tensor_mul(p[:], un3[:, :, 0], n3[:, :, 1])
    V.tensor_sub(r4[:, :, 0, 1], p[:], s3[:, :, 2])
    V.tensor_add(r4[:, :, 1, 0], p[:], s3[:, :, 2])
    V.tensor_mul(p[:], un3[:, :, 0], n3[:, :, 2])
    V.tensor_add(r4[:, :, 0, 2], p[:], s3[:, :, 1])
    V.tensor_sub(r4[:, :, 2, 0], p[:], s3[:, :, 1])
    V.tensor_mul(p[:], un3[:, :, 1], n3[:, :, 2])
    V.tensor_sub(r4[:, :, 1, 2], p[:], s3[:, :, 0])
    V.tensor_add(r4[:, :, 2, 1], p[:], s3[:, :, 0])


# ================================================================ entry

_CACHED = {}
DEBUG = False


def _get_nc():
    if "nc" not in _CACHED:
        _CACHED["nc"] = _build_nc()
    return _CACHED["nc"]


PROFILE = False


def kernel(**inputs):
    in_maps, vid_all = _host_prep(inputs)
    nc = _get_nc()
    res = run_bass_kernel_spmd(nc, in_maps, core_ids=list(range(NCORES)),
                               trace=PROFILE)
    _CACHED["last_res"] = res
    out = np.zeros((B, VS, 3), np.float32)
    for c in range(NCORES):
        o = np.asarray(res.results[c]["out"]).astype(np.float32).reshape(ROWS, 3, B)
        vok = vid_all[c] >= 0
        out[:, vid_all[c][vok], :] = o[vok].transpose(2, 0, 1)
    return out


# revision 42
# speedup vs baseline: 1.0295x; 1.0075x over previous
"""EHM (SMPLX body + FLAME head + MANO hands) Bass kernel for 8 TRN2 NeuronCores.

Sharding: VERTEX sharding -- model weights (shapedirs/posedirs/lbs weights)
dominate HBM traffic, so each core owns 1/8 of the SMPLX vertices (plus the
FLAME/MANO vertices its SMPLX rows stitch in) and computes ALL B=128 batch
elements for its shard.

v2 key restructure vs v1: joint regression is linear in betas, so
J = J_reg @ (template + shapedirs @ beta) is host-precomputed as
Jdirs = J_reg @ [shapedirs | template]  (tiny: 55x3x351).  On device the
joints come from a small Jdirs @ betaT matmul -- NO AllReduce, NO dependency
of FK on the big blend-shape stage.  FK (replicated, batch-on-partitions,
vector engine) fully overlaps the shapedirs/posedirs matmul stage.
Consequences: FLAME "even" chunks, MANO J chunks, smplx J regressor slabs all
vanish; head/hand chunks don't need smplx shapedirs (their rows get
overwritten by stitching); MANO rest joints are fully host-computed (betas
are batch-constant).

Per-vertex data layout: [vertex(partition<=128), (c, b)] with c-major free dim
(col = c*128 + b).  Batch-staged data (poses, FK, A matrices): [b(part), free].
"""

import sys

sys.path.insert(0, "/opt/trn_rl_repo")

from contextlib import ExitStack

import numpy as np
import ml_dtypes

BF16NP = ml_dtypes.bfloat16

import concourse.bass as bass
import concourse.bacc as bacc
import concourse.tile as tile
import concourse.mybir as mybir
from concourse.bass_utils import run_bass_kernel_spmd

F32 = mybir.dt.float32
BF16 = mybir.dt.bfloat16
AF = mybir.ActivationFunctionType
ALU = mybir.AluOpType

# ---------------------------------------------------------------- constants
B = 128
VS, VF, VM = 10475, 5023, 778
NL = 350
NCORES = 8

SMPLX_PARENTS = np.array([-1,0,0,0,1,2,3,4,5,6,7,8,9,9,9,12,13,14,16,17,18,19,
                          15,15,15,20,25,26,20,28,29,20,31,32,20,34,35,20,37,38,
                          21,40,41,21,43,44,21,46,47,21,49,50,21,52,53])
FLAME_PARENTS = np.array([-1,0,1,1,1])
MANO_PARENTS = np.array([-1,0,1,2,0,4,5,0,7,8,0,10,11,0,13,14])

N_PLAIN, N_HEAD, N_HL, N_HR = 768, 384, 128, 128
ROWS = N_PLAIN + N_HEAD + N_HL + N_HR        # 1408
NCH = ROWS // 128                            # 11
NCH_PLAIN = 6
CH_HEAD0 = 6                                 # chunks 6,7,8 head; 9 L; 10 R
CH_HL, CH_HR = 9, 10

PD_S_K = 189
PD_F_K = 27
PD_M_K = 135

NJ_ALL = 92
OFF_S, OFF_F, OFF_L, OFF_R = 0, 55, 60, 76
NROT = 55
ROT_S0, ROT_F0, ROT_L0, ROT_R0 = 0, 22, 25, 40

BF16_INPUTS = {"w_s", "wre_f", "w_m",
               "sd_s", "pd_s_a", "pd_s_b", "sd_f", "pd_f",
               "sd_m", "pd_m_a", "pd_m_b",
               "betaT_s", "betaT_f", "betam_rep", "jd_s", "jd_f"}


def _fk_forest():
    par = np.empty(NJ_ALL, np.int64)
    par[OFF_S:OFF_S + 55] = SMPLX_PARENTS
    par[OFF_F:OFF_F + 5] = np.where(FLAME_PARENTS < 0, -1, FLAME_PARENTS + OFF_F)
    par[OFF_L:OFF_L + 16] = np.where(MANO_PARENTS < 0, -1, MANO_PARENTS + OFF_L)
    par[OFF_R:OFF_R + 16] = np.where(MANO_PARENTS < 0, -1, MANO_PARENTS + OFF_R)
    return par


def _fk_levels(par):
    depth = np.zeros(NJ_ALL, np.int64)
    for j in range(NJ_ALL):
        if par[j] >= 0:
            depth[j] = depth[par[j]] + 1
    levels = []
    for d in range(1, int(depth.max()) + 1):
        js = np.nonzero(depth == d)[0]
        runs, i = [], 0
        while i < len(js):
            j0, p0 = int(js[i]), int(par[js[i]])
            if i + 1 < len(js):
                ds = int(js[i + 1]) - j0
                ps = int(par[js[i + 1]]) - p0
            else:
                ds, ps = 1, 0
            n = 1
            while (i + n < len(js) and int(js[i + n]) == j0 + n * ds
                   and int(par[js[i + n]]) == p0 + n * ps):
                n += 1
            if n == 1:
                ds, ps = 1, 0
            runs.append((j0, ds, n, p0, ps))
            i += n
        levels.append(runs)
    return levels


def _fk_levels_split():
    """Split forest levels into smplx-only runs and flame/mano runs (the trees
    are disjoint, so the smplx chain can be processed first)."""
    levels = _fk_levels(_fk_forest())
    ls, lfm = [], []
    for runs in levels:
        rs = [r for r in runs if r[0] < 55]
        rf = [r for r in runs if r[0] >= 55]
        if rs: ls.append(rs)
        if rf: lfm.append(rf)
    return ls, lfm


# ================================================================ host prep

def _split_sizes(total, parts):
    q, r = divmod(total, parts)
    return [q + (1 if i < r else 0) for i in range(parts)]


def _pad_ids(ids, n):
    out = np.full(n, -1, np.int64)
    out[:len(ids)] = ids
    return out


def _host_prep(inp):
    f32 = np.float32
    s2f = np.asarray(inp["smplx2flame_ind"])
    head_ix = np.asarray(inp["head_index"])
    s2l = np.asarray(inp["smplx2mano_left"])
    s2r = np.asarray(inp["smplx2mano_right"])

    head_sv = s2f[head_ix]
    special = np.zeros(VS, bool)
    special[head_sv] = True
    special[s2l] = True
    special[s2r] = True
    plain_sv = np.nonzero(~special)[0]

    pl_sp = np.cumsum([0] + _split_sizes(len(plain_sv), NCORES))
    hd_sp = np.cumsum([0] + _split_sizes(len(head_ix), NCORES))
    hl_sp = np.cumsum([0] + _split_sizes(VM, NCORES))

    sd_s_np = np.asarray(inp["smplx_shapedirs"], f32)
    pd_s_np = np.asarray(inp["smplx_posedirs"], f32)
    jr_s_np = np.asarray(inp["smplx_J_regressor"], f32)
    w_s_np = np.asarray(inp["smplx_lbs_weights"], f32)
    tmpl_s = np.asarray(inp["smplx_v_template"], f32)
    sd_f_np = np.asarray(inp["flame_shapedirs"], f32)
    pd_f_np = np.asarray(inp["flame_posedirs"], f32)
    jr_f_np = np.asarray(inp["flame_J_regressor"], f32)
    w_f_np = np.asarray(inp["flame_lbs_weights"], f32)
    tmpl_f = np.asarray(inp["flame_v_template"], f32)
    re_np = np.asarray(inp["r_eyelid"], f32)
    le_np = np.asarray(inp["l_eyelid"], f32)
    sd_m_np = np.asarray(inp["mano_shapedirs"], f32)
    pd_m_np = np.asarray(inp["mano_posedirs"], f32)
    jr_m_np = np.asarray(inp["mano_J_regressor"], f32)
    w_m_np = np.asarray(inp["mano_lbs_weights"], f32)
    tmpl_m = np.asarray(inp["mano_v_template"], f32)

    aa = np.concatenate([
        np.asarray(inp["global_pose"], f32).reshape(B, 3),
        np.asarray(inp["body_pose"], f32).reshape(B, 63),
        np.asarray(inp["jaw_params"], f32).reshape(B, 3),
        np.asarray(inp["eye_pose"], f32).reshape(B, 6),
        np.asarray(inp["left_hand_pose"], f32).reshape(B, 45),
        np.asarray(inp["right_hand_pose"], f32).reshape(B, 45),
    ], axis=1)

    # ---- MANO rest joints: batch-constant -> fully host-computed ----------
    betam_vec = np.asarray(inp["mano_betas"], f32)[0]           # [10]
    vshaped_m = tmpl_m + sd_m_np @ betam_vec                    # [VM, 3]
    J_m = jr_m_np @ vshaped_m                                   # [16, 3]
    rel_m = J_m.copy()
    rel_m[1:] -= J_m[MANO_PARENTS[1:]]
    jmb_rep = np.tile(np.ascontiguousarray(J_m.T).reshape(-1), (B, 1))
    relmb_rep = np.tile(np.ascontiguousarray(rel_m.T).reshape(-1), (B, 1))

    ep = np.asarray(inp["eyelid_params"], f32)
    aux = np.concatenate([
        np.asarray(inp["head_scale"], f32)[:, None],
        np.asarray(inp["left_hand_scale"], f32)[:, None],
        np.asarray(inp["right_hand_scale"], f32)[:, None],
        ep[:, 0:1], ep[:, 1:2],
        np.asarray(inp["head_pos_offset"], f32),
        np.asarray(inp["left_hand_pos_offset"], f32) - J_m[0][None],
        np.asarray(inp["right_hand_pos_offset"], f32) - J_m[0][None],
    ], axis=1)                                               # [128, 14]

    def beta_T(second):
        b = np.concatenate([np.asarray(inp["shape_params"], f32), second], 1)
        bt = np.zeros((384, B), f32)
        bt[:NL] = b.T
        bt[NL] = 1.0
        return bt.reshape(3, 128, B)

    betaT_s = beta_T(np.asarray(inp["body_exp"], f32))
    betaT_f = beta_T(np.asarray(inp["flame_exp"], f32))

    joff = np.asarray(inp["joints_offset"], f32)
    joffT = np.ascontiguousarray(joff.transpose(1, 2, 0)).reshape(55, 384)

    # ---- precomputed joint regressor directions: J = jd . [beta;1] --------
    def jdirs(jr, sd, tmpl, nj):
        ja = (jr @ sd.reshape(-1, 3 * NL)).reshape(nj, 3, NL)   # [nj,3,350]
        jt = jr @ tmpl                                          # [nj,3]
        out = np.zeros((3, 3, 128, nj), f32)
        for c in range(3):
            full = np.zeros((384, nj), f32)
            full[:NL] = ja[:, c, :].T
            full[NL] = jt[:, c]
            out[c] = full.reshape(3, 128, nj)
        return out.reshape(9, 128, nj)

    jd_s = jdirs(jr_s_np, sd_s_np, tmpl_s, 55)
    jd_f = jdirs(jr_f_np, sd_f_np, tmpl_f, 5)

    def mrel_T(par, nj):
        m = np.eye(nj, dtype=f32)
        for j in range(1, nj):
            if par[j] >= 0:
                m[j, par[j]] = -1.0
        return np.ascontiguousarray(m.T)

    betam_rep = np.zeros((11, 128), f32)
    betam_rep[:10] = betam_vec[:, None]
    betam_rep[10] = 1.0

    bpack = np.concatenate([aa, aux, jmb_rep, relmb_rep], 1)       # [128, 275]
    spack = np.zeros((55, 444), f32)
    spack[:, 0:55] = mrel_T(SMPLX_PARENTS, 55)
    spack[0:5, 55:60] = mrel_T(FLAME_PARENTS, 5)
    spack[:, 60:444] = joffT

    rep = dict(bpack=bpack, spack=spack, betaT_s=betaT_s, betaT_f=betaT_f,
               betam_rep=betam_rep, jd_s=jd_s, jd_f=jd_f,
               ident=np.eye(128, dtype=f32))

    in_maps = []
    vid_all = np.full((NCORES, ROWS), -1, np.int64)

    for c in range(NCORES):
        p_ids = plain_sv[pl_sp[c]:pl_sp[c + 1]]
        h_pos = np.arange(hd_sp[c], hd_sp[c + 1])
        h_sv, h_fv = head_sv[h_pos], head_ix[h_pos]
        l_pos = np.arange(hl_sp[c], hl_sp[c + 1])
        r_pos = l_pos                                         # same split for R
        l_sv, r_sv = s2l[l_pos], s2r[r_pos]

        vid = np.full(ROWS, -1, np.int64)
        vid[:len(p_ids)] = p_ids
        vid[N_PLAIN:N_PLAIN + len(h_sv)] = h_sv
        vid[N_PLAIN + N_HEAD:N_PLAIN + N_HEAD + len(l_sv)] = l_sv
        vid[N_PLAIN + N_HEAD + N_HL:N_PLAIN + N_HEAD + N_HL + len(r_sv)] = r_sv
        vid_all[c] = vid
        vok = vid >= 0
        vc = np.where(vok, vid, 0)

        # smplx shapedirs slab for PLAIN chunks only: [6, 128(p=l), (c, lk, v)]
        pv, pok = vc[:N_PLAIN], vok[:N_PLAIN]
        sdp = np.zeros((N_PLAIN, 3, 384), f32)
        sdp[:, :, :NL] = np.where(pok[:, None, None], sd_s_np[pv], 0.0)
        sdp[:, :, NL] = np.where(pok[:, None], tmpl_s[pv], 0.0)
        slab = sdp.reshape(NCH_PLAIN, 128, 3, 3, 128).transpose(0, 4, 2, 3, 1)
        sd_s = np.ascontiguousarray(slab).reshape(NCH_PLAIN, 128, 1152)

        colv = vc[:, None] * 3 + np.arange(3)[None, :]
        pdv = pd_s_np[:PD_S_K][:, colv]
        pdv = np.where(vok[None, :, None], pdv, 0.0)
        pdv = pdv.reshape(PD_S_K, NCH, 128, 3).transpose(1, 0, 3, 2)
        pd_s_a = np.ascontiguousarray(pdv[:, :128]).reshape(NCH, 128, 384)
        pd_s_b = np.ascontiguousarray(pdv[:, 128:]).reshape(NCH, PD_S_K - 128, 384)

        w_s = np.ascontiguousarray(
            np.where(vok[:, None], w_s_np[vc], 0.0)
            .reshape(NCH, 128, 55).transpose(0, 2, 1))

        # flame: 3 gathered chunks (only vertices actually stitched)
        fg = _pad_ids(h_fv, N_HEAD)
        fok = fg >= 0
        fc = np.where(fok, fg, 0)
        sdfp = np.zeros((N_HEAD, 3, 384), f32)
        sdfp[:, :, :NL] = np.where(fok[:, None, None], sd_f_np[fc], 0.0)
        sdfp[:, :, NL] = np.where(fok[:, None], tmpl_f[fc], 0.0)
        slab = sdfp.reshape(-1, 128, 3, 3, 128).transpose(0, 4, 2, 3, 1)
        sd_f = np.ascontiguousarray(slab).reshape(-1, 128, 1152)

        colf = fc[:, None] * 3 + np.arange(3)[None, :]
        pdfv = pd_f_np[9:36][:, colf]
        pdfv = np.where(fok[None, :, None], pdfv, 0.0)
        pdfv = pdfv.reshape(PD_F_K, 3, 128, 3).transpose(1, 0, 3, 2)
        pd_f = np.ascontiguousarray(pdfv).reshape(3, PD_F_K, 384)

        wre = np.zeros((3, 12, 128), f32)
        for k in range(3):
            rows, ok = fc[k * 128:(k + 1) * 128], fok[k * 128:(k + 1) * 128]
            wre[k, :5] = np.where(ok[None, :], w_f_np[rows].T, 0.0)
            wre[k, 5:8] = np.where(ok[None, :], re_np[rows].T, 0.0)
            wre[k, 8:11] = np.where(ok[None, :], le_np[rows].T, 0.0)
            wre[k, 11] = 1.0                                  # bias row

        # mano hands
        m_rows = np.stack([_pad_ids(l_pos, 128), _pad_ids(r_pos, 128)])
        mok = m_rows >= 0
        mc = np.where(mok, m_rows, 0)
        sd_m = np.zeros((2, 11, 384), f32)
        pd_m_a = np.zeros((2, 128, 384), f32)
        pd_m_b = np.zeros((2, PD_M_K - 128, 384), f32)
        w_m = np.zeros((2, 34, 128), f32)
        for h in range(2):
            sdm = np.where(mok[h][:, None, None], sd_m_np[mc[h]], 0.0)
            sd_m[h, :10] = sdm.transpose(2, 1, 0).reshape(10, 384)
            sd_m[h, 10] = np.where(mok[h][:, None], tmpl_m[mc[h]], 0.0).T.reshape(384)
            colm = mc[h][:, None] * 3 + np.arange(3)[None, :]
            pdm = pd_m_np[:, colm]
            pdm = np.where(mok[h][None, :, None], pdm, 0.0).transpose(0, 2, 1)
            pd_m_a[h] = pdm[:128].reshape(128, 384)
            pd_m_b[h] = pdm[128:].reshape(PD_M_K - 128, 384)
            w_m[h, h * 16:h * 16 + 16] = np.where(mok[h][None, :],
                                                  w_m_np[mc[h]].T, 0.0)
            w_m[h, 32 + h] = 1.0                              # bias row

        m = dict(rep)
        m.update(sd_s=sd_s, pd_s_a=pd_s_a, pd_s_b=pd_s_b, w_s=w_s,
                 sd_f=sd_f, pd_f=pd_f, wre_f=wre,
                 sd_m=sd_m, pd_m_a=pd_m_a, pd_m_b=pd_m_b, w_m=w_m)
        out = {}
        for k, v in m.items():
            if k in BF16_INPUTS:
                out[k] = np.ascontiguousarray(v.astype(BF16NP))
            else:
                out[k] = np.ascontiguousarray(v, f32)
        in_maps.append(out)

    return in_maps, vid_all


# ================================================================ device IR

def _build_nc():
    nc = bacc.Bacc("TRN2", target_bir_lowering=False, debug=False,
                   num_devices=NCORES)
    di = {}

    def din(name, shape):
        dt = BF16 if name in BF16_INPUTS else F32
        di[name] = nc.dram_tensor(name, list(shape), dt, kind="ExternalInput").ap()

    din("bpack", (B, 275)); din("spack", (55, 444))
    din("betaT_s", (3, 128, 128)); din("betaT_f", (3, 128, 128))
    din("betam_rep", (11, 128)); din("ident", (128, 128))
    din("jd_s", (9, 128, 55)); din("jd_f", (9, 128, 5))
    din("sd_s", (NCH_PLAIN, 128, 1152))
    din("pd_s_a", (NCH, 128, 384)); din("pd_s_b", (NCH, PD_S_K - 128, 384))
    din("w_s", (NCH, 55, 128))
    din("sd_f", (3, 128, 1152))
    din("pd_f", (3, PD_F_K, 384)); din("wre_f", (3, 12, 128))
    din("sd_m", (2, 11, 384)); din("pd_m_a", (2, 128, 384))
    din("pd_m_b", (2, PD_M_K - 128, 384)); din("w_m", (2, 34, 128))

    out_d = nc.dram_tensor("out", [ROWS, 384], BF16, kind="ExternalOutput").ap()
    dbg_d = None
    if DEBUG:
        dbg_d = nc.dram_tensor("dbg", [128, 8192], F32, kind="ExternalOutput").ap()

    with tile.TileContext(nc) as tc:
        _emit(nc, tc, di, out_d, dbg_d)
    nc.compile()
    return nc


def _emit(nc, tc, di, out_d, dbg_d=None):
    levels_s, levels_fm = _fk_levels_split()
    es = ExitStack()
    persist = es.enter_context(tc.tile_pool(name="persist", bufs=1))
    slabs = es.enter_context(tc.tile_pool(name="slabs", bufs=3))
    acc_cm = tc.tile_pool(name="acc", bufs=2, space="PSUM")
    acc = acc_cm.__enter__()
    big_cm = tc.tile_pool(name="big", bufs=2, space="PSUM")
    big = big_cm.__enter__()

    V, S, G, T, DMA = nc.vector, nc.scalar, nc.gpsimd, nc.tensor, nc.sync

    def ptile(shape, name):
        return persist.tile(list(shape), F32, tag=name, name=name)

    def btile(shape, name):
        return persist.tile(list(shape), BF16, tag=name, name=name)

    # ---------------- staged inputs ---------------------------------------
    # Each dma_start costs ~1.4us of issue time on its queue, so inputs are
    # packed into few transfers and spread over the three DGE queues
    # (Sync / Scalar / GpSimd) with the critical path (aa -> rodrigues,
    # jds/betaT -> joints -> FK) first on Sync.
    bpack = ptile((B, 275), "bpack")
    G.dma_start(bpack[:], di["bpack"][:])
    aa = bpack[:, 0:165]
    aux = bpack[:, 165:179]
    jmb = bpack[:, 179:227]
    relmb = bpack[:, 227:275]
    jds = btile((128, 9 * 55), "jds")
    DMA.dma_start(jds[:].rearrange("p (k j) -> p k j", j=55),
                  di["jd_s"][:].rearrange("k p j -> p k j"))
    betaT_s = btile((128, 384), "betaT_s")
    DMA.dma_start(betaT_s[:].rearrange("p (k b) -> p k b", b=128),
                  di["betaT_s"][:].rearrange("k p b -> p k b"))
    ident = ptile((128, 128), "ident")
    DMA.dma_start(ident[:], di["ident"][:])
    # flame-path inputs off the Sync queue (flame J is not start-critical)
    jdf = btile((128, 9 * 5), "jdf")
    S.dma_start(jdf[:].rearrange("p (k j) -> p k j", j=5),
                di["jd_f"][:].rearrange("k p j -> p k j"))
    betaT_f = btile((128, 384), "betaT_f")
    S.dma_start(betaT_f[:].rearrange("p (k b) -> p k b", b=128),
                di["betaT_f"][:].rearrange("k p b -> p k b"))
    spack = ptile((55, 444), "spack")
    G.dma_start(spack[:], di["spack"][:])
    mrelT_s = spack[:, 0:55]
    mrelT_f = spack[0:5, 55:60]
    joffT = spack[:, 60:444]
    betam = btile((11, 128), "betam")
    G.dma_start(betam[:], di["betam_rep"][:])

    # ---------------- rodrigues (V + S) -----------------------------------
    rot = ptile((B, NROT * 9), "rot")
    _rodrigues(nc, aa, rot, ptile)
    rot4 = rot[:].rearrange("p (j x) -> p j x", x=9)

    # skinning weights preloaded via the Scalar DGE queue (after rodrigues
    # so its activations are not delayed)
    w_all = persist.tile([55, NCH * 128], BF16, tag="w_all", name="w_all")
    S.dma_start(w_all[:].rearrange("j (i b) -> j i b", b=128),
                di["w_s"][:].rearrange("i j b -> j i b"))
    wre_all = persist.tile([12, 384], BF16, tag="wre_all", name="wre_all")
    S.dma_start(wre_all[:].rearrange("r (h b) -> r h b", b=128),
                di["wre_f"][:].rearrange("h r b -> r h b"))
    wm_all = persist.tile([34, 256], BF16, tag="wm_all", name="wm_all")
    S.dma_start(wm_all[:].rearrange("r (h b) -> r h b", b=128),
                di["w_m"][:].rearrange("h r b -> r h b"))

    # ---------------- joints from betas (tensor, tiny) ---------------------
    jp = acc.tile([128, 384], F32, tag="acc", padded_shape=[128, 512])
    for c3 in range(3):
        for lk in range(3):
            T.matmul(jp[0:55, c3 * 128:(c3 + 1) * 128],
                     jds[:, (c3 * 3 + lk) * 55:(c3 * 3 + lk + 1) * 55],
                     betaT_s[:, lk * 128:(lk + 1) * 128],
                     start=(lk == 0), stop=(lk == 2))
    tbj = ptile((55, 384), "tbj")
    V.tensor_add(tbj[:], jp[0:55, :], joffT)

    jpf = acc.tile([128, 384], F32, tag="acc", padded_shape=[128, 512])
    for c3 in range(3):
        for lk in range(3):
            T.matmul(jpf[0:5, c3 * 128:(c3 + 1) * 128],
                     jdf[:, (c3 * 3 + lk) * 5:(c3 * 3 + lk + 1) * 5],
                     betaT_f[:, lk * 128:(lk + 1) * 128],
                     start=(lk == 0), stop=(lk == 2))
    arr_f = ptile((5, 384), "arr_f")
    S.copy(arr_f[:], jpf[0:5, :])

    # rel joints
    rel_s = ptile((55, 384), "rel_s")
    pp = acc.tile([128, 384], F32, tag="acc", padded_shape=[128, 512])
    T.matmul(pp[0:55, :], mrelT_s, tbj[:], start=True, stop=True)
    S.copy(rel_s[:], pp[0:55, :])
    rel_f = ptile((5, 384), "rel_f")
    pp = acc.tile([128, 384], F32, tag="acc", padded_shape=[128, 512])
    T.matmul(pp[0:5, :], mrelT_f, arr_f[:], start=True, stop=True)
    S.copy(rel_f[:], pp[0:5, :])

    def transpose_to(dst_ap, src_ap):
        pq = acc.tile([128, 384], F32, tag="acc", padded_shape=[128, 512])
        k, n = src_ap.shape[0], src_ap.shape[1]
        T.matmul(pq[:n, :k], src_ap, ident[:k, :k], is_transpose=True,
                 start=True, stop=True)
        S.copy(dst_ap, pq[:n, :k])

    # batch-major staging of joints / rel for FK
    jb = ptile((B, 165), "jb")
    relb = ptile((B, 165), "relb")
    jfb = ptile((B, 15), "jfb")
    relfb = ptile((B, 15), "relfb")
    for c3 in range(3):
        transpose_to(jb[:, c3 * 55:(c3 + 1) * 55], tbj[:, c3 * 128:(c3 + 1) * 128])
        transpose_to(relb[:, c3 * 55:(c3 + 1) * 55], rel_s[:, c3 * 128:(c3 + 1) * 128])
        transpose_to(jfb[:, c3 * 5:(c3 + 1) * 5], arr_f[:, c3 * 128:(c3 + 1) * 128])
        transpose_to(relfb[:, c3 * 5:(c3 + 1) * 5], rel_f[:, c3 * 128:(c3 + 1) * 128])

    # pf = rot - I staged for posedirs matmuls (transposed, bf16)
    def pf_make(name, j0, n):
        t = ptile((B, n * 9), name)
        t9 = t[:].rearrange("p (j x) -> p j x", x=9)
        V.tensor_copy(t9, rot4[:, j0:j0 + n, :])
        V.tensor_scalar_add(t9[:, :, 0:9:4], t9[:, :, 0:9:4], -1.0)
        return t

    pf_s = pf_make("pf_s", 1, 21)
    pf_f = pf_make("pf_f", 22, 3)
    pf_m = [pf_make("pf_l", 25, 15), pf_make("pf_r", 40, 15)]

    pfT_s_a = btile((128, 128), "pfT_s_a")
    pfT_s_b = btile((PD_S_K - 128, 128), "pfT_s_b")
    transpose_to(pfT_s_a[:], pf_s[:, 0:128])
    transpose_to(pfT_s_b[:], pf_s[:, 128:PD_S_K])
    pfT_f = btile((PD_F_K, 128), "pfT_f")
    transpose_to(pfT_f[:], pf_f[:, :])
    pfT_m_a = [btile((128, 128), "pfT_l_a"), btile((128, 128), "pfT_r_a")]
    pfT_m_b = [btile((PD_M_K - 128, 128), "pfT_l_b"),
               btile((PD_M_K - 128, 128), "pfT_r_b")]
    for h in range(2):
        transpose_to(pfT_m_a[h][:], pf_m[h][:, 0:128])
        transpose_to(pfT_m_b[h][:], pf_m[h][:, 128:PD_M_K])

    # ---------------- FK (vector, batch on partitions) ---------------------
    Tb = ptile((B, NJ_ALL * 12), "Tb")
    Ab = ptile((B, NJ_ALL * 12), "Ab")
    T4 = Tb[:].rearrange("p (j m n) -> p j m n", m=3, n=4)
    A4 = Ab[:].rearrange("p (j m n) -> p j m n", m=3, n=4)
    G.memset(Tb[:], 0.0)
    for j0, n in ((22, 33), (OFF_F, 2), (OFF_L, 1), (OFF_R, 1)):
        G.memset(Tb[:].rearrange("p (j x) -> p j x", x=12)[:, j0:j0 + n, 0:11:5], 1.0)
    # rhs memsets done early while GpSimd is idle
    rhs_f = persist.tile([12, 1536], BF16, tag="rhs_f", name="rhs_f")
    G.memset(rhs_f[:], 0.0)
    rhs_lr = persist.tile([34, 1536], BF16, tag="rhs_lr", name="rhs_lr")
    G.memset(rhs_lr[:], 0.0)

    def rot_to_T(tj0, rj0, n):
        V.tensor_copy(T4[:, tj0:tj0 + n, :, 0:3],
                      rot4[:, rj0:rj0 + n, :].rearrange("p j (m n) -> p j m n", n=3))

    def fk_run(runs_list):
        fk_tmp2 = ptile((B, 12 * 16), "fk_tmp2")
        for runs in runs_list:
            for (d0, ds, n, p0, ps) in runs:
                sl_d = slice(d0, d0 + (n - 1) * ds + 1, ds) if ds != 1 else slice(d0, d0 + n)
                dst, dT = A4[:, sl_d], T4[:, sl_d]
                if ps == 0:
                    par = A4[:, p0:p0 + 1].broadcast_to([B, n, 3, 4])
                else:
                    sl_p = slice(p0, p0 + (n - 1) * ps + 1, ps) if ps != 1 else slice(p0, p0 + n)
                    par = A4[:, sl_p]
                sc2 = fk_tmp2[:].rearrange("p (j m n) -> p j m n", m=3, n=4)[:, :n]
                for k in range(3):
                    a_k = par[:, :, :, k:k + 1].broadcast_to([B, n, 3, 4])
                    t_k = dT[:, :, k:k + 1, :].broadcast_to([B, n, 3, 4])
                    if k == 0:
                        V.tensor_mul(dst, a_k, t_k)
                    else:
                        V.tensor_mul(sc2, a_k, t_k)
                        V.tensor_add(dst, dst, sc2)
                V.tensor_add(dst[:, :, :, 3], dst[:, :, :, 3], par[:, :, :, 3])

    corr_tmp = ptile((B, NJ_ALL * 3), "corr_tmp")
    corr_tmp2 = ptile((B, NJ_ALL * 3), "corr_tmp2")

    def corr(j0, nj, jsrc):
        ct = corr_tmp[:].rearrange("p (j m) -> p j m", m=3)[:, 0:nj]
        ct2 = corr_tmp2[:].rearrange("p (j m) -> p j m", m=3)[:, 0:nj]
        js = jsrc.rearrange("p (c j) -> p c j", c=3)
        for k in range(3):
            a_k = A4[:, j0:j0 + nj, :, k]
            j_k = js[:, k, :].unsqueeze(2).broadcast_to([B, nj, 3])
            if k == 0:
                V.tensor_mul(ct, a_k, j_k)
            else:
                V.tensor_mul(ct2, a_k, j_k)
                V.tensor_add(ct, ct, ct2)
        V.tensor_sub(A4[:, j0:j0 + nj, :, 3], A4[:, j0:j0 + nj, :, 3], ct)

    # ---- smplx chain first: fills, levels, corr --------------------------
    rot_to_T(0, ROT_S0, 22)
    V.tensor_copy(T4[:, 0:55, :, 3], relb[:].rearrange("p (c j) -> p j c", c=3))
    V.tensor_copy(A4[:, 0:1], T4[:, 0:1])
    fk_run(levels_s)
    corr(OFF_S, 55, jb[:])

    # ================= stage A part 1: plain + hand chunks =================
    vp_sbuf = [btile((128, 384), f"vp{i}") for i in range(NCH)]
    vpf_sbuf = [btile((128, 384), f"vpf{h}") for h in range(3)]
    vpm_sbuf = [btile((128, 384), f"vpm{h}") for h in range(2)]

    def copy_vp(dst_t, pq3):
        S.copy(dst_t[:, 0:384].rearrange("p (c b) -> p c b", b=128),
               pq3[:].rearrange("p (c x) -> p c x", x=512)[:, :, 0:128])

    def stage_a_chunk(i, pda=None, pdb=None):
        pq3 = big.tile([128, 1536], F32, tag="bigp")
        sdt = None
        if i < NCH_PLAIN:
            sdt = slabs.tile((128, 1152), BF16, tag="sd_s")
            DMA.dma_start(sdt[:], di["sd_s"][i])
        if pda is None:
            pda = slabs.tile((128, 384), BF16, tag="pd_s_a")
            pdb = slabs.tile((PD_S_K - 128, 384), BF16, tag="pd_s_b")
            DMA.dma_start(pda[:], di["pd_s_a"][i])
            DMA.dma_start(pdb[:], di["pd_s_b"][i])
        for c3 in range(3):
            r = slice(c3 * 512, c3 * 512 + 128)
            if sdt is not None:
                for lk in range(3):
                    T.matmul(pq3[:, r],
                             sdt[:, (c3 * 3 + lk) * 128:(c3 * 3 + lk + 1) * 128],
                             betaT_s[:, lk * 128:(lk + 1) * 128],
                             start=(lk == 0), stop=False)
        for c3 in range(3):
            r = slice(c3 * 512, c3 * 512 + 128)
            T.matmul(pq3[:, r], pda[:, c3 * 128:(c3 + 1) * 128], pfT_s_a[:],
                     start=(sdt is None), stop=False)
        for c3 in range(3):
            r = slice(c3 * 512, c3 * 512 + 128)
            T.matmul(pq3[:, r], pdb[:, c3 * 128:(c3 + 1) * 128], pfT_s_b[:],
                     start=False, stop=True)
        if i in (CH_HL, CH_HR):
            h = i - CH_HL
            sdm = slabs.tile((11, 384), BF16, tag="sd_m")
            DMA.dma_start(sdm[:], di["sd_m"][h])
            pma = slabs.tile((128, 384), BF16, tag="pd_m_a")
            pmb = slabs.tile((PD_M_K - 128, 384), BF16, tag="pd_m_b")
            DMA.dma_start(pma[:], di["pd_m_a"][h])
            DMA.dma_start(pmb[:], di["pd_m_b"][h])
            pq2 = big.tile([128, 1536], F32, tag="bigp")
            for c3 in range(3):
                T.matmul(pq2[:, c3 * 512:c3 * 512 + 128],
                         sdm[:, c3 * 128:(c3 + 1) * 128], betam[:],
                         start=True, stop=False)
            for c3 in range(3):
                T.matmul(pq2[:, c3 * 512:c3 * 512 + 128],
                         pma[:, c3 * 128:(c3 + 1) * 128], pfT_m_a[h][:],
                         start=False, stop=False)
            for c3 in range(3):
                T.matmul(pq2[:, c3 * 512:c3 * 512 + 128],
                         pmb[:, c3 * 128:(c3 + 1) * 128], pfT_m_b[h][:],
                         start=False, stop=True)
            copy_vp(vpm_sbuf[h], pq2)
        copy_vp(vp_sbuf[i], pq3)

    # preload head/flame slabs via the (idle) GpSimd DGE queue so those
    # matmuls aren't gated by the backed-up Sync queue
    sdf_t, pdf_t, hpd_a, hpd_b = [], [], [], []
    for h in range(3):
        t = persist.tile([128, 1152], BF16, tag=f"sdf{h}", name=f"sdf{h}")
        G.dma_start(t[:], di["sd_f"][h]); sdf_t.append(t)
        t = persist.tile([PD_F_K, 384], BF16, tag=f"pdf{h}", name=f"pdf{h}")
        G.dma_start(t[:], di["pd_f"][h]); pdf_t.append(t)
        i = CH_HEAD0 + h
        t = persist.tile([128, 384], BF16, tag=f"hpa{h}", name=f"hpa{h}")
        G.dma_start(t[:], di["pd_s_a"][i]); hpd_a.append(t)
        t = persist.tile([PD_S_K - 128, 384], BF16, tag=f"hpb{h}", name=f"hpb{h}")
        G.dma_start(t[:], di["pd_s_b"][i]); hpd_b.append(t)

    for i in list(range(NCH_PLAIN)) + [CH_HL, CH_HR]:
        stage_a_chunk(i)

    # ---- rhs_s (only needs the smplx chain) -------------------------------
    def rhs_fill(rhs_t, j0, nj):
        bp = big.tile([128, 1536], F32, tag="bigp")
        for n4 in range(4):
            for m3 in range(3):
                T.matmul(bp[0:nj, n4 * 384 + m3 * 128:n4 * 384 + (m3 + 1) * 128],
                         A4[:, j0:j0 + nj, m3, n4], ident[:],
                         is_transpose=True, start=True, stop=True)
        S.copy(rhs_t[0:nj, 0:1536], bp[0:nj, :])

    rhs_s = persist.tile([55, 1536], BF16, tag="rhs_s", name="rhs_s")
    rhs_fill(rhs_s, 0, 55)

    # head-pd chunks after the rhs_s fill: pulls the plain-chunk skinning
    # matmuls ~4us earlier on the tensor queue
    for h in range(3):
        stage_a_chunk(CH_HEAD0 + h, hpd_a[h], hpd_b[h])

    # ---- flame/mano chains ------------------------------------------------
    rot_to_T(OFF_F + 2, ROT_F0, 3)
    rot_to_T(OFF_L + 1, ROT_L0, 15)
    rot_to_T(OFF_R + 1, ROT_R0, 15)
    V.tensor_copy(T4[:, OFF_F:OFF_F + 5, :, 3],
                  relfb[:].rearrange("p (c j) -> p j c", c=3))
    for off in (OFF_L, OFF_R):
        V.tensor_copy(T4[:, off:off + 16, :, 3],
                      relmb.rearrange("p (c j) -> p j c", c=3))
    for r in (OFF_F, OFF_L, OFF_R):
        V.tensor_copy(A4[:, r:r + 1], T4[:, r:r + 1])
    fk_run(levels_fm)

    # ---- per-batch staging (world translations BEFORE rel-correction) ----
    hm = ptile((B, 16), "hm")
    jb3 = jb[:].rearrange("p (c j) -> p c j", c=3)
    bias9 = ptile((B, 9), "bias9")
    V.tensor_add(hm[:, 0:3], jb3[:, :, 23], jb3[:, :, 24])
    V.tensor_add(hm[:, 3:6], A4[:, OFF_F + 3, :, 3], A4[:, OFF_F + 4, :, 3])
    V.tensor_sub(hm[:, 6:9], hm[:, 0:3], hm[:, 3:6])
    V.scalar_tensor_tensor(bias9[:, 0:3], hm[:, 6:9], 0.5, aux[:, 5:8],
                           ALU.mult, ALU.add)
    V.tensor_sub(bias9[:, 3:4], jb3[:, 0:1, 20], aux[:, 8:9])
    V.tensor_add(bias9[:, 4:6], aux[:, 9:11], jb3[:, 1:3, 20])
    V.tensor_add(bias9[:, 6:9], aux[:, 11:14], jb3[:, :, 21])
    epp = ptile((B, 2), "epp")
    V.tensor_mul(epp[:], aux[:, 3:5], aux[:, 0:1].broadcast_to([B, 2]))

    corr(OFF_F, 5, jfb[:])
    corr(OFF_L, 16, jmb)
    corr(OFF_R, 16, jmb)

    # ---- scale folding --------------------------------------------------
    V.tensor_scalar_mul(Ab[:, OFF_F * 12:(OFF_F + 5) * 12],
                        Ab[:, OFF_F * 12:(OFF_F + 5) * 12], aux[:, 0:1])
    negls = ptile((B, 1), "negls")
    V.tensor_scalar_mul(negls[:], aux[:, 1:2], -1.0)
    AL = A4[:, OFF_L:OFF_L + 16]
    V.tensor_scalar_mul(AL[:, :, 0, :], AL[:, :, 0, :], negls[:, 0:1])
    V.tensor_scalar_mul(AL[:, :, 1:3, :], AL[:, :, 1:3, :], aux[:, 1:2])
    ARr = A4[:, OFF_R:OFF_R + 16]
    V.tensor_scalar_mul(ARr[:, :, :, :], ARr[:, :, :, :], aux[:, 2:3])

    # ================= skinning: plain chunks ==============================
    scr_t = [btile((128, 384), f"scr{i}") for i in range(4)]
    gscr = [btile((128, 384), f"gscr{i}") for i in range(2)]

    def t_apply(E, dst_ap, tp_ap, x_t, scratch):
        """dst = sum_{n<3} T'[n]*x_n + T'[3]; layouts (n, m, b)."""
        d3 = dst_ap.rearrange("p (m b) -> p m b", b=128)
        x3 = x_t[:, 0:384].rearrange("p (c b) -> p c b", b=128)
        tp = tp_ap.rearrange("p (n m b) -> p n m b", m=3, b=128)
        sc = scratch.rearrange("p (m b) -> p m b", b=128)
        E.tensor_mul(d3, tp[:, 0], x3[:, 0:1].broadcast_to([128, 3, 128]))
        for n4 in (1, 2):
            E.tensor_mul(sc, tp[:, n4], x3[:, n4:n4 + 1].broadcast_to([128, 3, 128]))
            E.tensor_add(d3, d3, sc)
        E.tensor_add(d3, d3, tp[:, 3])

    def skin_chunk(i):
        tps = big.tile([128, 1536], F32, tag="bigp")
        for g in range(3):
            T.matmul(tps[:, g * 512:(g + 1) * 512],
                     w_all[:, i * 128:(i + 1) * 128],
                     rhs_s[:, g * 512:(g + 1) * 512], start=True, stop=True)
        tpb = slabs.tile((128, 1536), BF16, tag="tpb", bufs=3, name="tpb")
        S.copy(tpb[:], tps[:])
        ot = slabs.tile((128, 384), BF16, tag="outt", bufs=3, name="ot")
        t_apply(V, ot[:], tpb[:], vp_sbuf[i], scr_t[i % 4][:])
        DMA.dma_start(out_d[i * 128:(i + 1) * 128, :], ot[:])

    for i in range(NCH_PLAIN):
        skin_chunk(i)

    # ================= rhs_f / rhs_m =======================================
    epT = persist.tile([2, 128], BF16, tag="epT", name="epT")
    transpose_to(epT[:], epp[:, :])
    bias9T = persist.tile([9, 128], BF16, tag="bias9T", name="bias9T")
    transpose_to(bias9T[:], bias9[:, :])
    rhs_fill(rhs_lr, OFF_L, 32)
    rhs_fill(rhs_f, OFF_F, 5)
    for m3 in range(3):
        G.dma_start(rhs_f[5 + m3:6 + m3, (9 + m3) * 128:(10 + m3) * 128],
                    epT[1:2, :])
        G.dma_start(rhs_f[8 + m3:9 + m3, (9 + m3) * 128:(10 + m3) * 128],
                    epT[0:1, :])
        G.dma_start(rhs_f[11:12, (9 + m3) * 128:(10 + m3) * 128],
                    bias9T[m3:m3 + 1, :])
    for h in range(2):
        for m3 in range(3):
            G.dma_start(rhs_lr[32 + h:33 + h, (9 + m3) * 128:(10 + m3) * 128],
                        bias9T[3 + 3 * h + m3:4 + 3 * h + m3, :])

    # ================= skinning: head + hand chunks ========================
    # pre-skin (flame/mano LBS) offloaded to GpSimd from a Scalar-copied
    # SBUF image of the PSUM tile, overlapping the Vector final applies
    def pre_skin(i):
        tpx = big.tile([128, 1536], F32, tag="bigp")
        if i < CH_HEAD0 + 3 and i >= CH_HEAD0:
            h = i - CH_HEAD0
            wsl, rhs_x, x_t = wre_all[:, h * 128:(h + 1) * 128], rhs_f, vpf_sbuf[h]
        else:
            h = i - CH_HL
            wsl, rhs_x, x_t = wm_all[:, h * 128:(h + 1) * 128], rhs_lr, vpm_sbuf[h]
        for g in range(3):
            T.matmul(tpx[:, g * 512:(g + 1) * 512], wsl,
                     rhs_x[:, g * 512:(g + 1) * 512], start=True, stop=True)
        tpxb = slabs.tile((128, 1536), BF16, tag="tpb", bufs=3, name="tpb")
        S.copy(tpxb[:], tpx[:])
        hv = slabs.tile((128, 384), BF16, tag="hv", bufs=2, name="hv")
        t_apply(V, hv[:], tpxb[:], x_t, gscr[i % 2][:])
        G.tensor_add(vp_sbuf[i][:, 0:384], vp_sbuf[i][:, 0:384], hv[:])
        skin_chunk(i)

    for i in (CH_HL, CH_HR):
        pre_skin(i)

    # flame stage-A (only gates the head chunks; runs while hands finish)
    for h in range(3):
        pq3 = big.tile([128, 1536], F32, tag="bigp")
        sdt = sdf_t[h]
        pdf = pdf_t[h]
        for c3 in range(3):
            r = slice(c3 * 512, c3 * 512 + 128)
            for lk in range(3):
                T.matmul(pq3[:, r],
                         sdt[:, (c3 * 3 + lk) * 128:(c3 * 3 + lk + 1) * 128],
                         betaT_f[:, lk * 128:(lk + 1) * 128],
                         start=(lk == 0), stop=False)
        for c3 in range(3):
            r = slice(c3 * 512, c3 * 512 + 128)
            T.matmul(pq3[:, r], pdf[:, c3 * 128:(c3 + 1) * 128], pfT_f[:],
                     start=False, stop=True)
        copy_vp(vpf_sbuf[h], pq3)

    for i in range(CH_HEAD0, CH_HEAD0 + 3):
        pre_skin(i)

    if dbg_d is not None:
        DMA.dma_start(dbg_d[0:128, 0:495], rot[:])
        DMA.dma_start(dbg_d[0:128, 512:1616], Ab[:])
        DMA.dma_start(dbg_d[0:128, 1664:1829], jb[:])
        DMA.dma_start(dbg_d[0:128, 1856:2021], relb[:])
        DMA.dma_start(dbg_d[0:128, 3200:3209], bias9[:])
        DMA.dma_start(dbg_d[0:128, 3216:3232], hm[:])
        DMA.dma_start(dbg_d[0:55, 3712:4096], tbj[:])
        DMA.dma_start(dbg_d[0:55, 4096:4480], rel_s[:])
        DMA.dma_start(dbg_d[0:5, 4480:4864], arr_f[:])
        DMA.dma_start(dbg_d[0:5, 4864:5248], rel_f[:])
        DMA.dma_start(dbg_d[0:128, 5376:6480], Tb[:])
    big_cm.__exit__(None, None, None)
    acc_cm.__exit__(None, None, None)
    es.close()


def _rodrigues(nc, aa, rot, ptile):
    V, S = nc.vector, nc.scalar
    J = NROT
    aa3 = aa[:].rearrange("p (j k) -> p j k", k=3)
    sq = ptile((B, J), "rg_sq")
    tmp = ptile((B, J), "rg_tmp")
    V.tensor_mul(sq[:], aa3[:, :, 0], aa3[:, :, 0])
    V.tensor_mul(tmp[:], aa3[:, :, 1], aa3[:, :, 1])
    V.tensor_add(sq[:], sq[:], tmp[:])
    V.tensor_mul(tmp[:], aa3[:, :, 2], aa3[:, :, 2])
    V.tensor_add(sq[:], sq[:], tmp[:])
    eps_t = ptile((B, 1), "rg_eps")
    nc.gpsimd.memset(eps_t[:], 1e-8)
    hpi_t = ptile((B, 1), "rg_hpi")
    nc.gpsimd.memset(hpi_t[:], float(np.pi / 2))
    zero_t = ptile((B, 1), "rg_zero")
    nc.gpsimd.memset(zero_t[:], 0.0)
    ang = ptile((B, J), "rg_ang")
    S.activation(ang[:], sq[:], AF.Sqrt, bias=eps_t[:])
    inv = ptile((B, J), "rg_inv")
    V.reciprocal(inv[:], ang[:])
    sn = ptile((B, J), "rg_sin")
    co = ptile((B, J), "rg_cos")
    S.activation(sn[:], ang[:], AF.Sin, bias=zero_t[:])
    S.activation(co[:], ang[:], AF.Sin, bias=hpi_t[:])
    nv = ptile((B, 3 * J), "rg_n")
    n3 = nv[:].rearrange("p (j k) -> p j k", k=3)
    V.tensor_mul(n3, aa3, inv[:].unsqueeze(2).broadcast_to([B, J, 3]))
    u = ptile((B, J), "rg_u")
    V.tensor_scalar(u[:], co[:], -1.0, 1.0, ALU.mult, ALU.add)
    un = ptile((B, 3 * J), "rg_un")
    un3 = un[:].rearrange("p (j k) -> p j k", k=3)
    V.tensor_mul(un3, n3, u[:].unsqueeze(2).broadcast_to([B, J, 3]))
    q = ptile((B, 3 * J), "rg_q")
    q3 = q[:].rearrange("p (j k) -> p j k", k=3)
    V.tensor_mul(q3, un3, n3)
    d = ptile((B, J), "rg_d")
    V.tensor_add(d[:], q3[:, :, 0], q3[:, :, 1])
    V.tensor_add(d[:], d[:], q3[:, :, 2])
    dd = ptile((B, J), "rg_dd")
    V.tensor_scalar(dd[:], d[:], -1.0, 1.0, ALU.mult, ALU.add)
    snv = ptile((B, 3 * J), "rg_snv")
    s3 = snv[:].rearrange("p (j k) -> p j k", k=3)
    V.tensor_mul(s3, n3, sn[:].unsqueeze(2).broadcast_to([B, J, 3]))
    r4 = rot[:].rearrange("p (j m n) -> p j m n", m=3, n=3)
    for m in range(3):
        V.tensor_add(r4[:, :, m, m], q3[:, :, m], dd[:])
    p = ptile((B, J), "rg_p")
    V.tensor_mul(p[:], un3[:, :, 0], n3[:, :, 1])
    V.tensor_sub(r4[:, :, 0, 1], p[:], s3[:, :, 2])
    V.tensor_add(r4[:, :, 1, 0], p[:], s3[:, :, 2])
    V.tensor_mul(p[:], un3[:, :, 0], n3[:, :, 2])
    V.tensor_add(r4[:, :, 0, 2], p[:], s3[:, :, 1])
    V.tensor_sub(r4[:, :, 2, 0], p[:], s3[:, :, 1])
    V.tensor_mul(p[:], un3[:, :, 1], n3[:, :, 2])
    V.tensor_sub(r4[:, :, 1, 2], p[:], s3[:, :, 0])
    V.tensor_add(r4[:, :, 2, 1], p[:], s3[:, :, 0])


# ================================================================ entry

_CACHED = {}
DEBUG = False


def _get_nc():
    if "nc" not in _CACHED:
        _CACHED["nc"] = _build_nc()
    return _CACHED["nc"]


PROFILE = False


def kernel(**inputs):
    in_maps, vid_all = _host_prep(inputs)
    nc = _get_nc()
    res = run_bass_kernel_spmd(nc, in_maps, core_ids=list(range(NCORES)),
                               trace=PROFILE)
    _CACHED["last_res"] = res
    out = np.zeros((B, VS, 3), np.float32)
    for c in range(NCORES):
        o = np.asarray(res.results[c]["out"]).astype(np.float32).reshape(ROWS, 3, B)
        vok = vid_all[c] >= 0
        out[:, vid_all[c][vok], :] = o[vok].transpose(2, 0, 1)
    return out
